# revision 1
# baseline (speedup 1.0000x reference)
"""AttentionPool (segment softmax-pool) Trainium2 kernel, 8 NeuronCores.

Math (reference):
    s = tanh(x @ W1 + b1) @ W2 + b2        # [N,1] scores
    e = exp(s - max(s))                    # global max shift
    out[b] = sum_{i in seg b} e_i x_i / (sum_{i in seg b} e_i + 1e-8)

The global max shift cancels in the ratio (|s| <= ||W2||_1 ~ 9 so exp
never overflows), so e = exp(s) directly.  Batch ids are sorted, so core c
owns segments [128c, 128(c+1)) and processes a fixed window of F rows
starting at the first row of segment 128c.  Rows outside the core's
segments self-mask: their relative id falls outside [0,128) so the
one-hot compare produces zero columns.

Device pipeline (per 512-row block, per core):
    u   = sum_g W18[:,g].T @dr xt8[:,g]   # fp8 DoubleRow matmuls, [H, 512]
    th  = tanh(u + b1)                    # ACT -> bf16
    ep  = w2b.T @ th                      # [1, 512] PSUM
    e   = exp(ep + b2)                    # ACT -> es1 row
per 24-tile group:
    e4  = PE-transpose each [1,128] slice of es1 -> [128, 24]  (departition)
per 128-row tile:
    A   = (iota == brel) * e              # DVE one-hot bf16
    num += A.T @ x_tile                   # bf16 moving, f32 PSUM
    den += A.T @ ones                     # 1-cycle matmul, [SEGS, 1]
Final: out = num / (den + 1e-8); host concat across cores.

Inputs are host-prepared: x in bf16 [F, 512] (pass 2), and a transposed
fp8-e4m3 copy xt8 [2, 128, 2, F] = x^T[g*256+k*128+p, c] for the DoubleRow
score matmuls (fp8 error only perturbs softmax weights; it largely cancels
in the num/den ratio).
"""

import os
import sys

for _p in ("/opt/trn_rl_repo",):
    if os.path.isdir(_p) and _p not in sys.path:
        sys.path.append(_p)

import numpy as np
import ml_dtypes

N_CORES = 8
B = 1024
SEGS = B // N_CORES          # 128 segments owned per core
D = 512
H = 128
F = 33792                    # fixed per-core row window (264 tiles of 128)
TILES = F // 128
GROUP = 8                    # tiles per DMA super-chunk / departition group
NGROUPS = TILES // GROUP     # 11


def build_nc(tiles=TILES, repeats=1, bufs=None, group=GROUP):
    """Build the per-core Bass program. repeats>1 re-emits the whole
    computation for delta-timing."""
    bufs = {**dict(x=3, x8=3, th=4, A=8, es=2, e4=2, u=2, ep=2),
            **(bufs or {})}
    import concourse.bacc as bacc
    import concourse.mybir as mybir
    import concourse.tile as tile

    F32 = mybir.dt.float32
    F32R = mybir.dt.float32r
    BF16 = mybir.dt.bfloat16
    FP8 = mybir.dt.float8e3
    FP8E4 = mybir.dt.float8e4
    DR = mybir.MatmulPerfMode.DoubleRow
    AF = mybir.ActivationFunctionType
    OP = mybir.AluOpType

    f_rows = tiles * 128
    ngroups = (tiles + group - 1) // group

    nc = bacc.Bacc(None, target_bir_lowering=False)
    xb_d = nc.dram_tensor("xb", (f_rows, 520), FP8, kind="ExternalInput")
    xt8_d = nc.dram_tensor("xt8", (128, 4, f_rows), FP8,
                           kind="ExternalInput")
    brl_d = nc.dram_tensor("brl", (128, tiles), F32, kind="ExternalInput")
    w18_d = nc.dram_tensor("w18", (128, 4, H), BF16, kind="ExternalInput")
    w2_d = nc.dram_tensor("w2", (H, 1), F32, kind="ExternalInput")
    b1_d = nc.dram_tensor("b1", (H, 1), F32, kind="ExternalInput")
    b2_d = nc.dram_tensor("b2", (128, 2), F32, kind="ExternalInput")
    out_d = nc.dram_tensor("out", (SEGS, D), F32, kind="ExternalOutput")

    import contextlib
    with tile.TileContext(nc) as tc:
        with contextlib.ExitStack() as _stk:
            cpool = _stk.enter_context(tc.tile_pool(name="const", bufs=1))
            xpool = _stk.enter_context(tc.tile_pool(name="xin", bufs=bufs["x"]))
            x8pool = _stk.enter_context(tc.tile_pool(name="x8in", bufs=bufs["x8"]))
            thpool = _stk.enter_context(tc.tile_pool(name="th", bufs=bufs["th"]))
            apool = _stk.enter_context(tc.tile_pool(name="abuild", bufs=bufs["A"]))
            espool = _stk.enter_context(tc.tile_pool(name="esb", bufs=bufs["es"]))
            e4pool = _stk.enter_context(tc.tile_pool(name="e4sb", bufs=bufs["e4"]))
            fpool = _stk.enter_context(tc.tile_pool(name="fin", bufs=1))
            upsum = _stk.enter_context(tc.tile_pool(name="ps_u", bufs=bufs["u"], space="PSUM"))
            eppsum = _stk.enter_context(tc.tile_pool(name="ps_ep", bufs=bufs["ep"], space="PSUM"))
            sppsum = _stk.enter_context(tc.tile_pool(name="ps_sp", bufs=2, space="PSUM"))
            eppsum = _stk.enter_context(tc.tile_pool(name="ps_ep", bufs=2, space="PSUM"))
            esbpool = _stk.enter_context(tc.tile_pool(name="esb", bufs=3))
            numpsum = _stk.enter_context(tc.tile_pool(name="ps_num", bufs=1, space="PSUM"))
            denpsum = _stk.enter_context(tc.tile_pool(name="ps_den", bufs=1, space="PSUM"))
            # ---- constants ----
            w18 = cpool.tile([128, 4, H], BF16)
            nc.scalar.dma_start(w18[:], w18_d[:])
            w2b = cpool.tile([H, 1], BF16)
            w2f = cpool.tile([H, 1], F32)
            nc.scalar.dma_start(w2f[:], w2_d[:])
            nc.vector.tensor_copy(w2b[:], w2f[:])
            b1s = cpool.tile([H, 1], F32)
            nc.scalar.dma_start(b1s[:], b1_d[:])
            b2both = cpool.tile([128, 2], F32)
            nc.scalar.dma_start(b2both[:], b2_d[:])
            b2s = b2both[:, 0:1]
            b2m8 = b2both[:, 1:2]
            brl = cpool.tile([128, tiles], F32)
            nc.scalar.dma_start(brl[:], brl_d[:])

            ii = cpool.tile([128, 128], mybir.dt.int32)
            nc.gpsimd.iota(ii[:], pattern=[[1, 128]], base=0,
                           channel_multiplier=0)
            iifb = cpool.tile([128, 128], BF16)
            nc.vector.tensor_copy(iifb[:], ii[:])
            i1 = cpool.tile([128, 1], mybir.dt.int32)
            nc.gpsimd.iota(i1[:], pattern=[[0, 1]], base=1,
                           channel_multiplier=0)
            onesb = cpool.tile([128, 1], BF16)
            nc.vector.tensor_copy(onesb[:], i1[:])
            # [1,1] identity (value 1.0) for the [1,128]->[128,1] transposes
            identb = cpool.tile([1, 1], BF16)
            nc.vector.tensor_copy(identb[:], i1[0:1, :])

            numa = numpsum.tile([SEGS, 260], F32)
            numb = denpsum.tile([SEGS, 256], F32)

            for rep in range(repeats):
                # Lag chains: score-mms/exp run one block behind u/tanh, A
                # builds two blocks behind, pass-2 matmuls one group behind.
                pend_sc = []      # blocks awaiting score-mm + exp
                pend_ab = []      # blocks awaiting A build
                pend_mm = []      # groups awaiting pass-2 matmuls
                groups_meta = {}  # s -> (sp, e4sb, is_dr, th-tiles, A-tiles)

                def build_sc(ent):
                    s0, b, gt, esb = ent
                    sp, e4sb, is_dr, thtiles, _ = groups_meta[s0]
                    t0, t1 = b * 4, min(b * 4 + 4, gt)
                    for t in range(t0, t1):
                        c0 = (t - t0) * 128
                        nc.tensor.transpose(
                            sp[:, t, 0:1],
                            esb[0:1, c0:c0 + 128],
                            identb[:],
                        )
                    nc.scalar.activation(
                        e4sb[:, t0:t1], sp[:, t0:t1, 0],
                        AF.Exp,
                        bias=(b2m8[:] if is_dr else b2s[:]), scale=1.0,
                    )

                def build_ab(ent):
                    s0, b, gt = ent
                    _, e4sb, is_dr, _, atiles = groups_meta[s0]
                    t0, t1 = b * 4, min(b * 4 + 4, gt)
                    for t in range(t0, t1):
                        T = s0 * group + t
                        A = apool.tile([128, SEGS], BF16, tag="A")
                        nc.vector.tensor_scalar(
                            A[:], iifb[:], brl[:, T:T + 1],
                            e4sb[:, t:t + 1],
                            op0=OP.is_equal, op1=OP.mult,
                        )
                        atiles[t] = A

                def flush(entry):
                    s0, xs_t, gt = entry
                    _, _, is_dr, _, atiles = groups_meta.pop(s0)
                    for t in range(gt):
                        T = s0 * group + t
                        A = atiles[t]
                        nc.tensor.matmul(
                            numa[:], A[:], xs_t[:, t, 0:260],
                            start=(T == 0), stop=(T == tiles - 1),
                            skip_group_check=True,
                        )
                        nc.tensor.matmul(
                            numb[:], A[:], xs_t[:, t, 260:516],
                            start=(T == 0), stop=(T == tiles - 1),
                            skip_group_check=True,
                        )

                def load_x8(s):
                    gt = min(group, tiles - s * group)
                    x8t = x8pool.tile([128, 4, group * 128], FP8, tag="x8")
                    nc.sync.dma_start(
                        x8t[:, :, 0:gt * 128],
                        xt8_d[:, :, s * group * 128:(s * group + gt) * 128],
                    )
                    return x8t

                x8next = load_x8(0)
                for s in range(ngroups):
                    gt = min(group, tiles - s * group)   # tiles this group
                    x8s = x8next
                    if s + 1 < ngroups:
                        x8next = load_x8(s + 1)
                    is_dr = False
                    xs = xpool.tile([128, group, 520], FP8, tag="x")
                    h = (gt + 1) // 2
                    for lo, hi in ((0, h), (h, gt)):
                        if hi > lo:
                            nc.sync.dma_start(
                                xs[:, lo:hi, :],
                                xb_d[(s * group + lo) * 128:
                                     (s * group + hi) * 128, :]
                                .rearrange("(q p) d -> p q d", p=128),
                            )
                    sp = sppsum.tile([128, group, 2], BF16, tag="sp")
                    e4sb = e4pool.tile([128, group], F32, tag="e4")
                    groups_meta[s] = (sp, e4sb, is_dr, {}, {})
                    nblk = (gt * 128 + 511) // 512
                    for b in range(nblk):
                        nb = min(512, gt * 128 - b * 512)
                        u = upsum.tile([H, 512], F32, tag="u")
                        for k in range(4):
                            nc.tensor.matmul(
                                u[:, 0:nb],
                                w18[:, k, :],
                                x8s[:, k, b * 512:b * 512 + nb],
                                start=(k == 0), stop=(k == 3),
                            )
                        th = thpool.tile([H, 512], BF16, tag="th")
                        nc.scalar.activation(
                            th[:, 0:nb], u[:, 0:nb],
                            AF.Tanh, bias=b1s[:], scale=1.0,
                        )
                        ep = eppsum.tile([1, 512], F32, tag="ep")
                        nc.tensor.matmul(
                            ep[:, 0:nb], w2b[:], th[:, 0:nb],
                            start=True, stop=True, skip_group_check=True,
                        )
                        esb = esbpool.tile([1, 512], BF16, tag="esb")
                        nc.vector.tensor_copy(esb[0:1, 0:nb], ep[0:1, 0:nb])
                        pend_sc.append((s, b, gt, esb))
                        if len(pend_sc) > 1:
                            build_sc(pend_sc.pop(0))
                        pend_ab.append((s, b, gt))
                        if len(pend_ab) > 2:
                            build_ab(pend_ab.pop(0))
                    pend_mm.append((s, xs, gt))
                    if len(pend_mm) > 1:
                        while pend_sc:
                            build_sc(pend_sc.pop(0))
                        while pend_ab:
                            build_ab(pend_ab.pop(0))
                        flush(pend_mm.pop(0))
                while pend_sc:
                    build_sc(pend_sc.pop(0))
                while pend_ab:
                    build_ab(pend_ab.pop(0))
                flush(pend_mm.pop(0))

                dsb = fpool.tile([SEGS, 1], F32, tag="dsb")
                nc.vector.tensor_scalar(dsb[:], numa[:, 256:257], 1e-8, None,
                                        op0=OP.add)
                rec = fpool.tile([SEGS, 1], F32, tag="rec")
                nc.vector.reciprocal(rec[:], dsb[:])
                osb = fpool.tile([SEGS, D], F32, tag="osb")
                nc.vector.tensor_scalar(osb[:, 0:256], numa[:, 0:256],
                                        rec[:], None, op0=OP.mult)
                nc.scalar.activation(osb[:, 256:512], numb[:, 0:256],
                                     AF.Copy, scale=rec[:])
                nc.sync.dma_start(out_d[:], osb[:])

    nc.compile()
    return nc


_NC_CACHE = {}


def get_nc(tiles=TILES):
    if tiles not in _NC_CACHE:
        _NC_CACHE[tiles] = build_nc(tiles)
    return _NC_CACHE[tiles]


def make_in_maps(x, batch, W1, b1, W2, b2, tiles=TILES, n_cores=N_CORES):
    """Host-side sharding: segment-aligned fixed windows + relative ids."""
    x = np.ascontiguousarray(np.asarray(x, dtype=np.float32))
    batch = np.asarray(batch).astype(np.int64)
    W1 = np.ascontiguousarray(np.asarray(W1, dtype=np.float32))
    b1 = np.asarray(b1, dtype=np.float32).reshape(H, 1)
    W2 = np.ascontiguousarray(np.asarray(W2, dtype=np.float32).reshape(H, 1))
    b2v = float(np.asarray(b2, dtype=np.float32).reshape(-1)[0])
    b2a = np.zeros((128, 2), np.float32)
    b2a[:, 0] = b2v
    b2a[:, 1] = b2v - np.log(8.0)
    fp8 = ml_dtypes.float8_e3m4

    # W18[p, k, h] = W1[k*128 + p, h]
    W18 = np.ascontiguousarray(
        W1.reshape(4, 128, H).transpose(1, 0, 2).astype(ml_dtypes.bfloat16))

    n = x.shape[0]
    f_rows = tiles * 128
    bounds = np.searchsorted(batch, np.arange(0, n_cores + 1) * SEGS)
    owned = np.diff(bounds)
    if owned.max() > f_rows:
        return None  # caller falls back
    pad_to = int(bounds[:-1].max() + f_rows)
    if pad_to > n:
        xp = np.concatenate([x, np.zeros((pad_to - n, D), np.float32)],
                            axis=0)
    else:
        xp = x
    in_maps = []
    for c in range(n_cores):
        o = int(bounds[c])
        xs = xp[o:o + f_rows]
        xa = np.zeros((f_rows, 520), np.float32)
        xa[:, 0:256] = xs[:, 0:256]
        xa[:, 256] = 1.0
        xa[:, 260:516] = xs[:, 256:512]
        xbb = np.ascontiguousarray(xa.astype(fp8))
        # xt8[g, p, k, c] = x^T[g*256 + k*128 + p, c]
        # xt8[p, k, c] = x^T[k*128 + p, c]
        xt8 = np.ascontiguousarray(
            xs.T.reshape(4, 128, f_rows).transpose(1, 0, 2).astype(fp8))
        nb = min(f_rows, n - o) if n > o else 0
        br = np.full(f_rows, -1.0, dtype=np.float32)
        br[:nb] = batch[o:o + nb].astype(np.float32) - c * SEGS
        brl2d = np.ascontiguousarray(
            br.reshape(tiles, 128).T)
        in_maps.append({
            "xb": xbb, "xt8": xt8, "brl": brl2d, "w18": W18,
            "w2": W2, "b1": b1, "b2": b2a,
        })
    return in_maps


def _numpy_fallback(x, batch, W1, b1, W2, b2):
    x = np.asarray(x, dtype=np.float32)
    batch = np.asarray(batch).astype(np.int64)
    scores = np.tanh(x @ W1 + b1) @ W2 + b2
    scores = scores - scores.max()
    e = np.exp(scores)
    den = np.zeros((B, 1), np.float32)
    np.add.at(den, batch, e)
    w = e / (den[batch] + 1e-8)
    out = np.zeros((B, D), np.float32)
    np.add.at(out, batch, w * x)
    return out


_RUNNER = {}


def _make_runner(nc, n_cores):
    """Reusable jitted SPMD executable (no donation) so repeated kernel()
    calls skip NEFF/XLA recompilation."""
    import jax
    import concourse.mybir as mybir
    from jax.sharding import Mesh, PartitionSpec, NamedSharding
    from jax.experimental.shard_map import shard_map
    from concourse import bass2jax

    bass2jax.install_neuronx_cc_hook()
    partition_name = (nc.partition_id_tensor.name
                      if nc.partition_id_tensor else None)
    in_names, out_names, out_avals, zero_outs = [], [], [], []
    for alloc in nc.m.functions[0].allocations:
        if not isinstance(alloc, mybir.MemoryLocationSet):
            continue
        name = alloc.memorylocations[0].name
        if alloc.kind == "ExternalInput":
            if name != partition_name:
                in_names.append(name)
        elif alloc.kind == "ExternalOutput":
            shape = tuple(alloc.tensor_shape)
            dtype = mybir.dt.np(alloc.dtype)
            out_names.append(name)
            out_avals.append(jax.core.ShapedArray(shape, dtype))
            zero_outs.append(np.zeros(shape, dtype))
    n_params = len(in_names)
    all_in_names = list(in_names) + list(out_names)
    if partition_name is not None:
        all_in_names.append(partition_name)

    def _body(*args):
        operands = list(args)
        if partition_name is not None:
            operands.append(bass2jax.partition_id_tensor())
        outs = bass2jax._bass_exec_p.bind(
            *operands,
            out_avals=tuple(out_avals),
            in_names=tuple(all_in_names),
            out_names=tuple(out_names),
            lowering_input_output_aliases=(),
            sim_require_finite=True,
            sim_require_nnan=True,
            nc=nc,
        )
        return tuple(outs)

    devices = jax.devices()[:n_cores]
    mesh = Mesh(np.asarray(devices), ("core",))
    nspec = n_params + len(out_names)
    fn = jax.jit(
        shard_map(_body, mesh=mesh,
                  in_specs=(PartitionSpec("core"),) * nspec,
                  out_specs=(PartitionSpec("core"),) * len(out_names),
                  check_rep=False),
        keep_unused=True,
    )
    sharding = NamedSharding(mesh, PartitionSpec("core"))
    concat_zero = [
        np.zeros((n_cores * z.shape[0], *z.shape[1:]), z.dtype)
        for z in zero_outs
    ]
    zero_dev = [jax.device_put(a, sharding) for a in concat_zero]
    return dict(fn=fn, in_names=in_names, out_names=out_names,
                out_avals=out_avals, zero_dev=zero_dev, sharding=sharding)


def _run_fast(nc, in_maps, n_cores):
    import jax
    key = id(nc)
    if key not in _RUNNER:
        _RUNNER[key] = _make_runner(nc, n_cores)
    r = _RUNNER[key]
    concat_in = [
        np.concatenate([np.asarray(in_maps[c][name]) for c in range(n_cores)],
                       axis=0)
        for name in r["in_names"]
    ]
    dev_in = [jax.device_put(a, r["sharding"]) for a in concat_in]
    outs = r["fn"](*dev_in, *r["zero_dev"])
    jax.block_until_ready(outs)
    return [
        {name: np.asarray(outs[i]).reshape(n_cores, *r["out_avals"][i].shape)[c]
         for i, name in enumerate(r["out_names"])}
        for c in range(n_cores)
    ]


def kernel(x, batch, W1, b1, W2, b2):
    x = np.asarray(x)
    batch = np.asarray(batch)
    if (x.shape != (262144, D) or batch.shape != (262144,)
            or np.asarray(W1).shape != (D, H)):
        return _numpy_fallback(x, batch, W1, b1, W2, b2)
    if np.any(batch[:-1] > batch[1:]):
        return _numpy_fallback(x, batch, W1, b1, W2, b2)
    b64 = batch.astype(np.int64)
    bounds = np.searchsorted(b64, np.arange(0, N_CORES + 1) * SEGS)
    owned_max = int(np.diff(bounds).max())
    tiles = max(GROUP, -(-owned_max // 128))
    in_maps = make_in_maps(x, batch, W1, b1, W2, b2, tiles=tiles)
    if in_maps is None:
        return _numpy_fallback(x, batch, W1, b1, W2, b2)
    nc = get_nc(tiles)
    try:
        res = _run_fast(nc, in_maps, N_CORES)
        return np.concatenate([res[c]["out"] for c in range(N_CORES)], axis=0)
    except Exception:
        from concourse.bass_utils import run_bass_kernel_spmd
        res = run_bass_kernel_spmd(nc, in_maps, list(range(N_CORES)))
        return np.concatenate(
            [res.results[c]["out"] for c in range(N_CORES)], axis=0)


if __name__ == "__main__":
    pass



# revision 3
# speedup vs baseline: 1.0785x; 1.0785x over previous
"""AttentionPool (segment softmax-pool) Trainium2 kernel, 8 NeuronCores.

Math (reference):
    s = tanh(x @ W1 + b1) @ W2 + b2        # [N,1] scores
    e = exp(s - max(s))                    # global max shift
    out[b] = sum_{i in seg b} e_i x_i / (sum_{i in seg b} e_i + 1e-8)

The global max shift cancels in the ratio (|s| <= ||W2||_1 ~ 9 so exp
never overflows), so e = exp(s) directly.  Batch ids are sorted, so core c
owns segments [128c, 128(c+1)) and processes a fixed window of F rows
starting at the first row of segment 128c.  Rows outside the core's
segments self-mask: their relative id falls outside [0,128) so the
one-hot compare produces zero columns.

Device pipeline (per 16-tile / 4-block group, per core):
  per 512-row block b:
    u   = sum_k W18[:,k].T @ x8[:,k]      # fp8 matmuls, [H, 512] PSUM
    th  = tanh(u + b1)                    # ACT -> bf16
  per group (lagged one group):
    ep4[32b] = w2b.T @ th_b               # 4 col-tiled M=1 matmuls at
                                          # tile_position (0,32b): disjoint
                                          # PE col-groups -> concurrent on HW
    esb = ep4[::32]                       # ONE strided 4-partition DVE copy
    sp  = PE-transpose each [1,128] slice -> [128, gt]  (departition)
    e4  = exp(sp + b2)                    # ACT
    A[t] = (iota == brel[t]) * e4[t]      # DVE one-hot bf16
  per group (lagged two groups), per tile:
    num += A.T @ x_tile                   # fp8 moving, f32 PSUM (260+256)
Final: out = num / (den + 1e-8); host concat across cores.
den rides along as x column 256 == 1.0.

Inputs are host-prepared fp8-e3m4: xb [F, 520] row-major (pass 2) and a
transposed copy xt8 [128, 4, F] (score matmuls).  fp8 error on scores
largely cancels in the softmax ratio; on the pooled sum it stays below
the 2e-2 gate.
"""

import os
import sys

for _p in ("/opt/trn_rl_repo",):
    if os.path.isdir(_p) and _p not in sys.path:
        sys.path.append(_p)

import numpy as np
import ml_dtypes

N_CORES = 8
B = 1024
SEGS = B // N_CORES          # 128 segments owned per core
D = 512
H = 128
F = 33792                    # fixed per-core row window (264 tiles of 128)
TILES = F // 128
GROUP = 16                   # tiles per DMA chunk / departition group
NGROUPS = -(-TILES // GROUP)


def build_nc(tiles=TILES, repeats=1, bufs=None, group=GROUP):
    """Build the per-core Bass program. repeats>1 re-emits the whole
    computation for delta-timing."""
    bufs = {**dict(x=3, x8=3, th=8, A=2, esb=2, e4=2, u=2, ep=2, sp=2),
            **(bufs or {})}
    import concourse.bacc as bacc
    import concourse.mybir as mybir
    import concourse.tile as tile

    F32 = mybir.dt.float32
    BF16 = mybir.dt.bfloat16
    FP8 = mybir.dt.float8e3
    AF = mybir.ActivationFunctionType
    OP = mybir.AluOpType

    f_rows = tiles * 128
    ngroups = (tiles + group - 1) // group

    nc = bacc.Bacc(None, target_bir_lowering=False)
    xb_d = nc.dram_tensor("xb", (f_rows, 520), FP8, kind="ExternalInput")
    xt8_d = nc.dram_tensor("xt8", (128, 4, f_rows), FP8,
                           kind="ExternalInput")
    brl_d = nc.dram_tensor("brl", (128, tiles), F32, kind="ExternalInput")
    w18_d = nc.dram_tensor("w18", (128, 4, H), BF16, kind="ExternalInput")
    w2_d = nc.dram_tensor("w2", (H, 1), F32, kind="ExternalInput")
    b1_d = nc.dram_tensor("b1", (H, 1), F32, kind="ExternalInput")
    b2_d = nc.dram_tensor("b2", (128, 1), F32, kind="ExternalInput")
    out_d = nc.dram_tensor("out", (SEGS, D), F32, kind="ExternalOutput")

    import contextlib
    with tile.TileContext(nc) as tc:
        with contextlib.ExitStack() as _stk:
            cpool = _stk.enter_context(tc.tile_pool(name="const", bufs=1))
            xpool = _stk.enter_context(tc.tile_pool(name="xin", bufs=bufs["x"]))
            x8pool = _stk.enter_context(tc.tile_pool(name="x8in", bufs=bufs["x8"]))
            thpool = _stk.enter_context(tc.tile_pool(name="th", bufs=bufs["th"]))
            apool = _stk.enter_context(tc.tile_pool(name="abuild", bufs=bufs["A"]))
            esbpool = _stk.enter_context(tc.tile_pool(name="esb", bufs=bufs["esb"]))
            e4pool = _stk.enter_context(tc.tile_pool(name="e4sb", bufs=bufs["e4"]))
            fpool = _stk.enter_context(tc.tile_pool(name="fin", bufs=1))
            upsum = _stk.enter_context(tc.tile_pool(name="ps_u", bufs=bufs["u"], space="PSUM"))
            eppsum = _stk.enter_context(tc.tile_pool(name="ps_ep", bufs=bufs["ep"], space="PSUM"))
            sppsum = _stk.enter_context(tc.tile_pool(name="ps_sp", bufs=bufs["sp"], space="PSUM"))
            numpsum = _stk.enter_context(tc.tile_pool(name="ps_num", bufs=1, space="PSUM"))
            denpsum = _stk.enter_context(tc.tile_pool(name="ps_den", bufs=1, space="PSUM"))
            # ---- constants ----
            w18 = cpool.tile([128, 4, H], BF16)
            nc.scalar.dma_start(w18[:], w18_d[:])
            w2b = cpool.tile([H, 1], BF16)
            w2f = cpool.tile([H, 1], F32)
            nc.scalar.dma_start(w2f[:], w2_d[:])
            nc.vector.tensor_copy(w2b[:], w2f[:])
            b1s = cpool.tile([H, 1], F32)
            nc.scalar.dma_start(b1s[:], b1_d[:])
            b2s = cpool.tile([128, 1], F32)
            nc.scalar.dma_start(b2s[:], b2_d[:])
            brl = cpool.tile([128, tiles], F32)
            nc.scalar.dma_start(brl[:], brl_d[:])

            ii = cpool.tile([128, 128], mybir.dt.int32)
            nc.gpsimd.iota(ii[:], pattern=[[1, 128]], base=0,
                           channel_multiplier=0)
            iifb = cpool.tile([128, 128], BF16)
            nc.vector.tensor_copy(iifb[:], ii[:])
            i1 = cpool.tile([128, 1], mybir.dt.int32)
            nc.gpsimd.iota(i1[:], pattern=[[0, 1]], base=1,
                           channel_multiplier=0)
            # [128,1] ones in bf16: transpose identity (sliced per block row)
            ones128 = cpool.tile([128, 1], BF16)
            nc.vector.tensor_copy(ones128[:], i1[:])

            numa = numpsum.tile([SEGS, 260], F32)
            numb = denpsum.tile([SEGS, 256], F32)

            for rep in range(repeats):
                # Stage queues (each holds at most one group):
                #   Qep: blocks awaiting the 4 col-tiled ep matmuls + esb copy
                #   Qdp: groups awaiting departition transposes + exp
                #   Qab: groups awaiting A build
                #   Qmm: groups awaiting pass-2 matmuls (depth 2)
                Qep, Qdp, Qab, Qmm = [], [], [], []

                def flush_ep(ent):
                    s, ep4, ths = ent
                    for b, th, nb in ths:
                        nc.tensor.matmul(
                            ep4[32 * b:32 * b + 1, 0:nb],
                            w2b[:], th[:, 0:nb],
                            start=True, stop=True, skip_group_check=True,
                            tile_position=(0, 32 * b),
                        )
                    esb4 = esbpool.tile([128, 512], BF16, tag="esb")
                    full = all(nb == 512 for _, _, nb in ths)
                    if full:
                        # contiguous copy (PSUM forbids partition step>1);
                        # rows between the 32-strided score rows are stale
                        # PSUM and never read downstream
                        nc.vector.tensor_copy(esb4[:, :], ep4[:, :])
                    else:
                        for b, _, nb in ths:
                            nc.vector.tensor_copy(
                                esb4[32 * b:32 * b + 1, 0:nb],
                                ep4[32 * b:32 * b + 1, 0:nb])
                    Qdp.append((s, esb4))

                def flush_dp(ent):
                    s, esb4 = ent
                    gt = min(group, tiles - s * group)
                    sp = sppsum.tile([128, group, 2], BF16, tag="sp")
                    for t in range(gt):
                        j = t // 4
                        c0 = (t % 4) * 128
                        nc.tensor.transpose(
                            sp[:, t, 0:1],
                            esb4[32 * j:32 * j + 1, c0:c0 + 128],
                            ones128[32 * j:32 * j + 1, :],
                            tile_position=(32 * j, 0),
                        )
                    e4sb = e4pool.tile([128, group], F32, tag="e4")
                    nc.scalar.activation(
                        e4sb[:, 0:gt], sp[:, 0:gt, 0],
                        AF.Exp, bias=b2s[:], scale=1.0,
                    )
                    Qab.append((s, e4sb))

                def flush_ab(ent):
                    s, e4sb = ent
                    gt = min(group, tiles - s * group)
                    Ag = apool.tile([128, group, SEGS], BF16, tag="A")
                    for t in range(gt):
                        T = s * group + t
                        nc.vector.tensor_scalar(
                            Ag[:, t, :], iifb[:], brl[:, T:T + 1],
                            e4sb[:, t:t + 1],
                            op0=OP.is_equal, op1=OP.mult,
                        )
                    for i, ent2 in enumerate(Qmm):
                        if ent2[0] == s:
                            Qmm[i] = (s, ent2[1], Ag)

                def flush_mm(ent):
                    s, xs_t, Ag = ent
                    gt = min(group, tiles - s * group)
                    for t in range(gt):
                        T = s * group + t
                        nc.tensor.matmul(
                            numa[:], Ag[:, t, :], xs_t[:, t, 0:260],
                            start=(T == 0), stop=(T == tiles - 1),
                            skip_group_check=True,
                        )
                        nc.tensor.matmul(
                            numb[:], Ag[:, t, :], xs_t[:, t, 260:516],
                            start=(T == 0), stop=(T == tiles - 1),
                            skip_group_check=True,
                        )

                def load_x8(s):
                    gt = min(group, tiles - s * group)
                    x8t = x8pool.tile([128, 4, group * 128], FP8, tag="x8")
                    nc.sync.dma_start(
                        x8t[:, :, 0:gt * 128],
                        xt8_d[:, :, s * group * 128:(s * group + gt) * 128],
                    )
                    return x8t

                x8next = load_x8(0)
                for s in range(ngroups):
                    gt = min(group, tiles - s * group)   # tiles this group
                    x8s = x8next
                    if s + 1 < ngroups:
                        x8next = load_x8(s + 1)
                    xs = xpool.tile([128, group, 520], FP8, tag="x")
                    h = (gt + 1) // 2
                    for lo, hi in ((0, h), (h, gt)):
                        if hi > lo:
                            nc.sync.dma_start(
                                xs[:, lo:hi, :],
                                xb_d[(s * group + lo) * 128:
                                     (s * group + hi) * 128, :]
                                .rearrange("(q p) d -> p q d", p=128),
                            )
                    ep4 = eppsum.tile([128, 512], F32, tag="ep")
                    ths = []
                    nblk = (gt * 128 + 511) // 512
                    for b in range(nblk):
                        nb = min(512, gt * 128 - b * 512)
                        u = upsum.tile([H, 512], F32, tag="u")
                        for k in range(4):
                            nc.tensor.matmul(
                                u[:, 0:nb],
                                w18[:, k, :],
                                x8s[:, k, b * 512:b * 512 + nb],
                                start=(k == 0), stop=(k == 3),
                            )
                        th = thpool.tile([H, 512], BF16, tag="th")
                        nc.scalar.activation(
                            th[:, 0:nb], u[:, 0:nb],
                            AF.Tanh, bias=b1s[:], scale=1.0,
                        )
                        ths.append((b, th, nb))
                        if b == 0 and Qep:
                            flush_ep(Qep.pop(0))
                        if b == 1 and Qdp:
                            flush_dp(Qdp.pop(0))
                        if b == 2 and Qab:
                            flush_ab(Qab.pop(0))
                    # partial groups may not hit all flush points
                    if nblk <= 2 and Qab:
                        flush_ab(Qab.pop(0))
                    if nblk <= 1 and Qdp:
                        flush_dp(Qdp.pop(0))
                    Qep.append((s, ep4, ths))
                    Qmm.append((s, xs, None))
                    if len(Qmm) > 2:
                        ent = Qmm.pop(0)
                        assert ent[2] is not None
                        flush_mm(ent)
                # drain
                while Qep:
                    flush_ep(Qep.pop(0))
                while Qdp:
                    flush_dp(Qdp.pop(0))
                while Qab:
                    flush_ab(Qab.pop(0))
                while Qmm:
                    ent = Qmm.pop(0)
                    assert ent[2] is not None
                    flush_mm(ent)

                dsb = fpool.tile([SEGS, 1], F32, tag="dsb")
                nc.vector.tensor_scalar(dsb[:], numa[:, 256:257], 1e-8, None,
                                        op0=OP.add)
                rec = fpool.tile([SEGS, 1], F32, tag="rec")
                nc.vector.reciprocal(rec[:], dsb[:])
                osb = fpool.tile([SEGS, D], F32, tag="osb")
                nc.vector.tensor_scalar(osb[:, 0:256], numa[:, 0:256],
                                        rec[:], None, op0=OP.mult)
                nc.scalar.activation(osb[:, 256:512], numb[:, 0:256],
                                     AF.Copy, scale=rec[:])
                nc.sync.dma_start(out_d[:], osb[:])

    nc.compile()
    return nc


_NC_CACHE = {}


def get_nc(tiles=TILES):
    if tiles not in _NC_CACHE:
        _NC_CACHE[tiles] = build_nc(tiles)
    return _NC_CACHE[tiles]


def make_in_maps(x, batch, W1, b1, W2, b2, tiles=TILES, n_cores=N_CORES):
    """Host-side sharding: segment-aligned fixed windows + relative ids."""
    x = np.ascontiguousarray(np.asarray(x, dtype=np.float32))
    batch = np.asarray(batch).astype(np.int64)
    W1 = np.ascontiguousarray(np.asarray(W1, dtype=np.float32))
    b1 = np.asarray(b1, dtype=np.float32).reshape(H, 1)
    W2 = np.ascontiguousarray(np.asarray(W2, dtype=np.float32).reshape(H, 1))
    b2v = float(np.asarray(b2, dtype=np.float32).reshape(-1)[0])
    b2a = np.full((128, 1), b2v, np.float32)
    fp8 = ml_dtypes.float8_e3m4

    # W18[p, k, h] = W1[k*128 + p, h]
    W18 = np.ascontiguousarray(
        W1.reshape(4, 128, H).transpose(1, 0, 2).astype(ml_dtypes.bfloat16))

    n = x.shape[0]
    f_rows = tiles * 128
    bounds = np.searchsorted(batch, np.arange(0, n_cores + 1) * SEGS)
    owned = np.diff(bounds)
    if owned.max() > f_rows:
        return None  # caller falls back
    pad_to = int(bounds[:-1].max() + f_rows)
    if pad_to > n:
        xp = np.concatenate([x, np.zeros((pad_to - n, D), np.float32)],
                            axis=0)
    else:
        xp = x
    in_maps = []
    for c in range(n_cores):
        o = int(bounds[c])
        xs = xp[o:o + f_rows]
        xa = np.zeros((f_rows, 520), np.float32)
        xa[:, 0:256] = xs[:, 0:256]
        xa[:, 256] = 1.0
        xa[:, 260:516] = xs[:, 256:512]
        xbb = np.ascontiguousarray(xa.astype(fp8))
        # xt8[p, k, c] = x^T[k*128 + p, c]
        xt8 = np.ascontiguousarray(
            xs.T.reshape(4, 128, f_rows).transpose(1, 0, 2).astype(fp8))
        nb = min(f_rows, n - o) if n > o else 0
        br = np.full(f_rows, -1.0, dtype=np.float32)
        br[:nb] = batch[o:o + nb].astype(np.float32) - c * SEGS
        brl2d = np.ascontiguousarray(
            br.reshape(tiles, 128).T)
        in_maps.append({
            "xb": xbb, "xt8": xt8, "brl": brl2d, "w18": W18,
            "w2": W2, "b1": b1, "b2": b2a,
        })
    return in_maps


def _numpy_fallback(x, batch, W1, b1, W2, b2):
    x = np.asarray(x, dtype=np.float32)
    batch = np.asarray(batch).astype(np.int64)
    scores = np.tanh(x @ W1 + b1) @ W2 + b2
    scores = scores - scores.max()
    e = np.exp(scores)
    den = np.zeros((B, 1), np.float32)
    np.add.at(den, batch, e)
    w = e / (den[batch] + 1e-8)
    out = np.zeros((B, D), np.float32)
    np.add.at(out, batch, w * x)
    return out


_RUNNER = {}


def _make_runner(nc, n_cores):
    """Reusable jitted SPMD executable (no donation) so repeated kernel()
    calls skip NEFF/XLA recompilation."""
    import jax
    import concourse.mybir as mybir
    from jax.sharding import Mesh, PartitionSpec, NamedSharding
    from jax.experimental.shard_map import shard_map
    from concourse import bass2jax

    bass2jax.install_neuronx_cc_hook()
    partition_name = (nc.partition_id_tensor.name
                      if nc.partition_id_tensor else None)
    in_names, out_names, out_avals, zero_outs = [], [], [], []
    for alloc in nc.m.functions[0].allocations:
        if not isinstance(alloc, mybir.MemoryLocationSet):
            continue
        name = alloc.memorylocations[0].name
        if alloc.kind == "ExternalInput":
            if name != partition_name:
                in_names.append(name)
        elif alloc.kind == "ExternalOutput":
            shape = tuple(alloc.tensor_shape)
            dtype = mybir.dt.np(alloc.dtype)
            out_names.append(name)
            out_avals.append(jax.core.ShapedArray(shape, dtype))
            zero_outs.append(np.zeros(shape, dtype))
    n_params = len(in_names)
    all_in_names = list(in_names) + list(out_names)
    if partition_name is not None:
        all_in_names.append(partition_name)

    def _body(*args):
        operands = list(args)
        if partition_name is not None:
            operands.append(bass2jax.partition_id_tensor())
        outs = bass2jax._bass_exec_p.bind(
            *operands,
            out_avals=tuple(out_avals),
            in_names=tuple(all_in_names),
            out_names=tuple(out_names),
            lowering_input_output_aliases=(),
            sim_require_finite=True,
            sim_require_nnan=True,
            nc=nc,
        )
        return tuple(outs)

    devices = jax.devices()[:n_cores]
    mesh = Mesh(np.asarray(devices), ("core",))
    nspec = n_params + len(out_names)
    fn = jax.jit(
        shard_map(_body, mesh=mesh,
                  in_specs=(PartitionSpec("core"),) * nspec,
                  out_specs=(PartitionSpec("core"),) * len(out_names),
                  check_rep=False),
        keep_unused=True,
    )
    sharding = NamedSharding(mesh, PartitionSpec("core"))
    concat_zero = [
        np.zeros((n_cores * z.shape[0], *z.shape[1:]), z.dtype)
        for z in zero_outs
    ]
    zero_dev = [jax.device_put(a, sharding) for a in concat_zero]
    return dict(fn=fn, in_names=in_names, out_names=out_names,
                out_avals=out_avals, zero_dev=zero_dev, sharding=sharding)


def _run_fast(nc, in_maps, n_cores):
    import jax
    key = id(nc)
    if key not in _RUNNER:
        _RUNNER[key] = _make_runner(nc, n_cores)
    r = _RUNNER[key]
    concat_in = [
        np.concatenate([np.asarray(in_maps[c][name]) for c in range(n_cores)],
                       axis=0)
        for name in r["in_names"]
    ]
    dev_in = [jax.device_put(a, r["sharding"]) for a in concat_in]
    outs = r["fn"](*dev_in, *r["zero_dev"])
    jax.block_until_ready(outs)
    return [
        {name: np.asarray(outs[i]).reshape(n_cores, *r["out_avals"][i].shape)[c]
         for i, name in enumerate(r["out_names"])}
        for c in range(n_cores)
    ]


def kernel(x, batch, W1, b1, W2, b2):
    x = np.asarray(x)
    batch = np.asarray(batch)
    if (x.shape != (262144, D) or batch.shape != (262144,)
            or np.asarray(W1).shape != (D, H)):
        return _numpy_fallback(x, batch, W1, b1, W2, b2)
    if np.any(batch[:-1] > batch[1:]):
        return _numpy_fallback(x, batch, W1, b1, W2, b2)
    b64 = batch.astype(np.int64)
    bounds = np.searchsorted(b64, np.arange(0, N_CORES + 1) * SEGS)
    owned_max = int(np.diff(bounds).max())
    tiles = max(GROUP, -(-owned_max // 128))
    in_maps = make_in_maps(x, batch, W1, b1, W2, b2, tiles=tiles)
    if in_maps is None:
        return _numpy_fallback(x, batch, W1, b1, W2, b2)
    nc = get_nc(tiles)
    try:
        res = _run_fast(nc, in_maps, N_CORES)
        return np.concatenate([res[c]["out"] for c in range(N_CORES)], axis=0)
    except Exception:
        from concourse.bass_utils import run_bass_kernel_spmd
        res = run_bass_kernel_spmd(nc, in_maps, list(range(N_CORES)))
        return np.concatenate(
            [res.results[c]["out"] for c in range(N_CORES)], axis=0)


if __name__ == "__main__":
    pass


# revision 5
# speedup vs baseline: 1.2813x; 1.1880x over previous
"""AttentionPool (segment softmax-pool) Trainium2 kernel, 8 NeuronCores.

Math (reference):
    s = tanh(x @ W1 + b1) @ W2 + b2        # [N,1] scores
    e = exp(s - max(s))                    # global max shift
    out[b] = sum_{i in seg b} e_i x_i / (sum_{i in seg b} e_i + 1e-8)

The global max shift cancels in the ratio (|s| <= ||W2||_1 ~ 9 so exp
never overflows), so e = exp(s) directly.  Batch ids are sorted, so core c
owns segments [128c, 128(c+1)) and processes a fixed window of F rows
starting at the first row of segment 128c.  Rows outside the core's
segments self-mask: their relative id falls outside [0,128) so the
one-hot compare produces zero columns.

Device pipeline (per 16-tile / 4-block group, per core):
  per 512-row block b:
    u   = sum_k W18[:,k].T @ x8[:,k]      # fp8 matmuls, [H, 512] PSUM
    th  = tanh(u + b1)                    # ACT -> bf16
  per group (lagged one group):
    ep4[32b] = w2b.T @ th_b               # 4 col-tiled M=1 matmuls at
                                          # tile_position (0,32b): disjoint
                                          # PE col-groups -> concurrent on HW
    esb = ep4[::32]                       # ONE strided 4-partition DVE copy
    sp  = PE-transpose each [1,128] slice -> [128, gt]  (departition)
    e4  = exp(sp + b2)                    # ACT
    A[t] = (iota == brel[t]) * e4[t]      # DVE one-hot bf16
  per group (lagged two groups), per tile:
    num += A.T @ x_tile                   # fp8 moving, f32 PSUM (260+256)
Final: out = num / (den + 1e-8); host concat across cores.
den rides along as x column 256 == 1.0.

Inputs are host-prepared fp8-e3m4: xb [F, 520] row-major (pass 2) and a
transposed copy xt8 [128, 4, F] (score matmuls).  fp8 error on scores
largely cancels in the softmax ratio; on the pooled sum it stays below
the 2e-2 gate.
"""

import os
import sys

for _p in ("/opt/trn_rl_repo",):
    if os.path.isdir(_p) and _p not in sys.path:
        sys.path.append(_p)

import numpy as np
import ml_dtypes

N_CORES = 8
B = 1024
SEGS = B // N_CORES          # 128 segments owned per core
D = 512
H = 128
F = 33792                    # fixed per-core row window (264 tiles of 128)
TILES = F // 128
GROUP = 16                   # tiles per DMA chunk / departition group
NGROUPS = -(-TILES // GROUP)


def build_nc(tiles=TILES, repeats=1, bufs=None, group=GROUP):
    """Build the per-core Bass program. repeats>1 re-emits the whole
    computation for delta-timing."""
    bufs = {**dict(x=3, x8=3, th=8, A=2, esb=2, e4=2, u=2, ep=2, sp=2),
            **(bufs or {})}
    import concourse.bacc as bacc
    import concourse.mybir as mybir
    import concourse.tile as tile

    F32 = mybir.dt.float32
    BF16 = mybir.dt.bfloat16
    FP8 = mybir.dt.float8e3
    AF = mybir.ActivationFunctionType
    OP = mybir.AluOpType

    f_rows = tiles * 128
    ngroups = (tiles + group - 1) // group

    nc = bacc.Bacc(None, target_bir_lowering=False)
    xb_d = nc.dram_tensor("xb", (f_rows, 520), FP8, kind="ExternalInput")
    xt8_d = nc.dram_tensor("xt8", (128, 4, f_rows), FP8,
                           kind="ExternalInput")
    brl_d = nc.dram_tensor("brl", (128, tiles), F32, kind="ExternalInput")
    w18_d = nc.dram_tensor("w18", (128, 4, H), BF16, kind="ExternalInput")
    w2_d = nc.dram_tensor("w2", (H, 1), F32, kind="ExternalInput")
    b1_d = nc.dram_tensor("b1", (H, 1), F32, kind="ExternalInput")
    b2_d = nc.dram_tensor("b2", (128, 1), F32, kind="ExternalInput")
    out_d = nc.dram_tensor("out", (SEGS, D), F32, kind="ExternalOutput")

    import contextlib
    with tile.TileContext(nc) as tc:
        with contextlib.ExitStack() as _stk:
            cpool = _stk.enter_context(tc.tile_pool(name="const", bufs=1))
            xpool = _stk.enter_context(tc.tile_pool(name="xin", bufs=bufs["x"]))
            x8pool = _stk.enter_context(tc.tile_pool(name="x8in", bufs=bufs["x8"]))
            thpool = _stk.enter_context(tc.tile_pool(name="th", bufs=bufs["th"]))
            apool = _stk.enter_context(tc.tile_pool(name="abuild", bufs=bufs["A"]))
            esbpool = _stk.enter_context(tc.tile_pool(name="esb", bufs=bufs["esb"]))
            e4pool = _stk.enter_context(tc.tile_pool(name="e4sb", bufs=bufs["e4"]))
            fpool = _stk.enter_context(tc.tile_pool(name="fin", bufs=1))
            upsum = _stk.enter_context(tc.tile_pool(name="ps_u", bufs=bufs["u"], space="PSUM"))
            eppsum = _stk.enter_context(tc.tile_pool(name="ps_ep", bufs=bufs["ep"], space="PSUM"))
            sppsum = _stk.enter_context(tc.tile_pool(name="ps_sp", bufs=bufs["sp"], space="PSUM"))
            numpsum = _stk.enter_context(tc.tile_pool(name="ps_num", bufs=1, space="PSUM"))
            denpsum = _stk.enter_context(tc.tile_pool(name="ps_den", bufs=1, space="PSUM"))
            # ---- constants ----
            w18 = cpool.tile([128, 4, H], BF16)
            nc.scalar.dma_start(w18[:], w18_d[:])
            w2b = cpool.tile([H, 1], BF16)
            w2f = cpool.tile([H, 1], F32)
            nc.scalar.dma_start(w2f[:], w2_d[:])
            nc.vector.tensor_copy(w2b[:], w2f[:])
            b1s = cpool.tile([H, 1], F32)
            nc.scalar.dma_start(b1s[:], b1_d[:])
            b2s = cpool.tile([128, 1], F32)
            nc.scalar.dma_start(b2s[:], b2_d[:])
            brl = cpool.tile([128, tiles], F32)
            nc.scalar.dma_start(brl[:], brl_d[:])

            ii = cpool.tile([128, 128], mybir.dt.int32)
            nc.gpsimd.iota(ii[:], pattern=[[1, 128]], base=0,
                           channel_multiplier=0)
            iifb = cpool.tile([128, 128], BF16)
            nc.vector.tensor_copy(iifb[:], ii[:])
            i1 = cpool.tile([128, 1], mybir.dt.int32)
            nc.gpsimd.iota(i1[:], pattern=[[0, 1]], base=1,
                           channel_multiplier=0)
            # [128,1] ones in bf16: transpose identity (sliced per block row)
            ones128 = cpool.tile([128, 1], BF16)
            nc.vector.tensor_copy(ones128[:], i1[:])

            numa = numpsum.tile([SEGS, 260], F32)
            numb = denpsum.tile([SEGS, 256], F32)

            for rep in range(repeats):
                # Stage queues (each holds at most one group):
                #   Qep: blocks awaiting the 4 col-tiled ep matmuls + esb copy
                #   Qdp: groups awaiting departition transposes + exp
                #   Qab: groups awaiting A build
                #   Qmm: groups awaiting pass-2 matmuls (depth 2)
                Qep, Qdp, Qab, Qmm = [], [], [], []

                def flush_ep(ent):
                    s, ep4, ths = ent
                    for b, th, nb in ths:
                        nc.tensor.matmul(
                            ep4[32 * b:32 * b + 1, 0:nb],
                            w2b[:], th[:, 0:nb],
                            start=True, stop=True, skip_group_check=True,
                            tile_position=(0, 32 * b),
                        )
                    esb4 = esbpool.tile([128, 512], BF16, tag="esb")
                    full = all(nb == 512 for _, _, nb in ths)
                    if full:
                        # contiguous copy (PSUM forbids partition step>1);
                        # rows between the 32-strided score rows are stale
                        # PSUM and never read downstream
                        nc.vector.tensor_copy(esb4[:, :], ep4[:, :])
                    else:
                        for b, _, nb in ths:
                            nc.vector.tensor_copy(
                                esb4[32 * b:32 * b + 1, 0:nb],
                                ep4[32 * b:32 * b + 1, 0:nb])
                    Qdp.append((s, esb4))

                def flush_dp(ent):
                    s, esb4 = ent
                    gt = min(group, tiles - s * group)
                    sp = sppsum.tile([128, group, 2], BF16, tag="sp")
                    for t in range(gt):
                        j = t // 4
                        c0 = (t % 4) * 128
                        nc.tensor.transpose(
                            sp[:, t, 0:1],
                            esb4[32 * j:32 * j + 1, c0:c0 + 128],
                            ones128[32 * j:32 * j + 1, :],
                            tile_position=(32 * j, 0),
                        )
                    e4sb = e4pool.tile([128, group], F32, tag="e4")
                    nc.scalar.activation(
                        e4sb[:, 0:gt], sp[:, 0:gt, 0],
                        AF.Exp, bias=b2s[:], scale=1.0,
                    )
                    Qab.append((s, e4sb))

                def flush_ab(ent):
                    s, e4sb = ent
                    gt = min(group, tiles - s * group)
                    Ag = apool.tile([128, group, SEGS], BF16, tag="A")
                    for t in range(gt):
                        T = s * group + t
                        nc.vector.tensor_scalar(
                            Ag[:, t, :], iifb[:], brl[:, T:T + 1],
                            e4sb[:, t:t + 1],
                            op0=OP.is_equal, op1=OP.mult,
                        )
                    for i, ent2 in enumerate(Qmm):
                        if ent2[0] == s:
                            Qmm[i] = (s, ent2[1], Ag)

                def flush_mm(ent):
                    s, xs_t, Ag = ent
                    gt = min(group, tiles - s * group)
                    for t in range(gt):
                        T = s * group + t
                        nc.tensor.matmul(
                            numa[:], Ag[:, t, :], xs_t[:, t, 0:260],
                            start=(T == 0), stop=(T == tiles - 1),
                            skip_group_check=True,
                        )
                        nc.tensor.matmul(
                            numb[:], Ag[:, t, :], xs_t[:, t, 260:516],
                            start=(T == 0), stop=(T == tiles - 1),
                            skip_group_check=True,
                        )

                def load_x8(s, split=1):
                    gt = min(group, tiles - s * group)
                    x8t = x8pool.tile([128, 4, group * 128], FP8, tag="x8")
                    c = gt * 128
                    step = -(-c // split)
                    for lo in range(0, c, step):
                        hi = min(lo + step, c)
                        nc.sync.dma_start(
                            x8t[:, :, lo:hi],
                            xt8_d[:, :, s * group * 128 + lo:
                                  s * group * 128 + hi],
                        )
                    return x8t

                x8next = load_x8(0, split=4)
                for s in range(ngroups):
                    gt = min(group, tiles - s * group)   # tiles this group
                    x8s = x8next
                    if s + 1 < ngroups:
                        x8next = load_x8(s + 1)
                    xs = xpool.tile([128, group, 520], FP8, tag="x")
                    h = (gt + 1) // 2
                    for lo, hi in ((0, h), (h, gt)):
                        if hi > lo:
                            nc.sync.dma_start(
                                xs[:, lo:hi, :],
                                xb_d[(s * group + lo) * 128:
                                     (s * group + hi) * 128, :]
                                .rearrange("(q p) d -> p q d", p=128),
                            )
                    ep4 = eppsum.tile([128, 512], F32, tag="ep")
                    ths = []
                    nblk = (gt * 128 + 511) // 512
                    for b in range(nblk):
                        nb = min(512, gt * 128 - b * 512)
                        u = upsum.tile([H, 512], F32, tag="u")
                        for k in range(4):
                            nc.tensor.matmul(
                                u[:, 0:nb],
                                w18[:, k, :],
                                x8s[:, k, b * 512:b * 512 + nb],
                                start=(k == 0), stop=(k == 3),
                            )
                        th = thpool.tile([H, 512], BF16, tag="th")
                        nc.scalar.activation(
                            th[:, 0:nb], u[:, 0:nb],
                            AF.Tanh, bias=b1s[:], scale=1.0,
                        )
                        ths.append((b, th, nb))
                        if b == 0 and Qep:
                            flush_ep(Qep.pop(0))
                        if b == 1 and Qdp:
                            flush_dp(Qdp.pop(0))
                        if b == 2 and Qab:
                            flush_ab(Qab.pop(0))
                    # partial groups may not hit all flush points
                    if nblk <= 2 and Qab:
                        flush_ab(Qab.pop(0))
                    if nblk <= 1 and Qdp:
                        flush_dp(Qdp.pop(0))
                    Qep.append((s, ep4, ths))
                    Qmm.append((s, xs, None))
                    if len(Qmm) > 2:
                        ent = Qmm.pop(0)
                        assert ent[2] is not None
                        flush_mm(ent)
                # drain
                while Qep:
                    flush_ep(Qep.pop(0))
                while Qdp:
                    flush_dp(Qdp.pop(0))
                while Qab:
                    flush_ab(Qab.pop(0))
                while Qmm:
                    ent = Qmm.pop(0)
                    assert ent[2] is not None
                    flush_mm(ent)

                dsb = fpool.tile([SEGS, 1], F32, tag="dsb")
                nc.vector.tensor_scalar(dsb[:], numa[:, 256:257], 1e-8, None,
                                        op0=OP.add)
                rec = fpool.tile([SEGS, 1], F32, tag="rec")
                nc.vector.reciprocal(rec[:], dsb[:])
                osb = fpool.tile([SEGS, D], F32, tag="osb")
                nc.vector.tensor_scalar(osb[:, 0:256], numa[:, 0:256],
                                        rec[:], None, op0=OP.mult)
                nc.scalar.activation(osb[:, 256:512], numb[:, 0:256],
                                     AF.Copy, scale=rec[:])
                nc.sync.dma_start(out_d[:], osb[:])

    nc.compile()
    return nc


_NC_CACHE = {}


def get_nc(tiles=TILES):
    if tiles not in _NC_CACHE:
        _NC_CACHE[tiles] = build_nc(tiles)
    return _NC_CACHE[tiles]


def make_in_maps(x, batch, W1, b1, W2, b2, tiles=TILES, n_cores=N_CORES):
    """Host-side sharding: segment-aligned fixed windows + relative ids."""
    x = np.ascontiguousarray(np.asarray(x, dtype=np.float32))
    batch = np.asarray(batch).astype(np.int64)
    W1 = np.ascontiguousarray(np.asarray(W1, dtype=np.float32))
    b1 = np.asarray(b1, dtype=np.float32).reshape(H, 1)
    W2 = np.ascontiguousarray(np.asarray(W2, dtype=np.float32).reshape(H, 1))
    b2v = float(np.asarray(b2, dtype=np.float32).reshape(-1)[0])
    b2a = np.full((128, 1), b2v, np.float32)
    fp8 = ml_dtypes.float8_e3m4

    # W18[p, k, h] = W1[k*128 + p, h]
    W18 = np.ascontiguousarray(
        W1.reshape(4, 128, H).transpose(1, 0, 2).astype(ml_dtypes.bfloat16))

    n = x.shape[0]
    f_rows = tiles * 128
    bounds = np.searchsorted(batch, np.arange(0, n_cores + 1) * SEGS)
    owned = np.diff(bounds)
    if owned.max() > f_rows:
        return None  # caller falls back
    pad_to = int(bounds[:-1].max() + f_rows)
    if pad_to > n:
        xp = np.concatenate([x, np.zeros((pad_to - n, D), np.float32)],
                            axis=0)
    else:
        xp = x
    in_maps = []
    for c in range(n_cores):
        o = int(bounds[c])
        xs = xp[o:o + f_rows]
        xa = np.zeros((f_rows, 520), np.float32)
        xa[:, 0:256] = xs[:, 0:256]
        xa[:, 256] = 1.0
        xa[:, 260:516] = xs[:, 256:512]
        xbb = np.ascontiguousarray(xa.astype(fp8))
        # xt8[p, k, c] = x^T[k*128 + p, c]
        xt8 = np.ascontiguousarray(
            xs.T.reshape(4, 128, f_rows).transpose(1, 0, 2).astype(fp8))
        nb = min(f_rows, n - o) if n > o else 0
        br = np.full(f_rows, -1.0, dtype=np.float32)
        br[:nb] = batch[o:o + nb].astype(np.float32) - c * SEGS
        brl2d = np.ascontiguousarray(
            br.reshape(tiles, 128).T)
        in_maps.append({
            "xb": xbb, "xt8": xt8, "brl": brl2d, "w18": W18,
            "w2": W2, "b1": b1, "b2": b2a,
        })
    return in_maps


def _numpy_fallback(x, batch, W1, b1, W2, b2):
    x = np.asarray(x, dtype=np.float32)
    batch = np.asarray(batch).astype(np.int64)
    scores = np.tanh(x @ W1 + b1) @ W2 + b2
    scores = scores - scores.max()
    e = np.exp(scores)
    den = np.zeros((B, 1), np.float32)
    np.add.at(den, batch, e)
    w = e / (den[batch] + 1e-8)
    out = np.zeros((B, D), np.float32)
    np.add.at(out, batch, w * x)
    return out


_RUNNER = {}


def _make_runner(nc, n_cores):
    """Reusable jitted SPMD executable (no donation) so repeated kernel()
    calls skip NEFF/XLA recompilation."""
    import jax
    import concourse.mybir as mybir
    from jax.sharding import Mesh, PartitionSpec, NamedSharding
    from jax.experimental.shard_map import shard_map
    from concourse import bass2jax

    bass2jax.install_neuronx_cc_hook()
    partition_name = (nc.partition_id_tensor.name
                      if nc.partition_id_tensor else None)
    in_names, out_names, out_avals, zero_outs = [], [], [], []
    for alloc in nc.m.functions[0].allocations:
        if not isinstance(alloc, mybir.MemoryLocationSet):
            continue
        name = alloc.memorylocations[0].name
        if alloc.kind == "ExternalInput":
            if name != partition_name:
                in_names.append(name)
        elif alloc.kind == "ExternalOutput":
            shape = tuple(alloc.tensor_shape)
            dtype = mybir.dt.np(alloc.dtype)
            out_names.append(name)
            out_avals.append(jax.core.ShapedArray(shape, dtype))
            zero_outs.append(np.zeros(shape, dtype))
    n_params = len(in_names)
    all_in_names = list(in_names) + list(out_names)
    if partition_name is not None:
        all_in_names.append(partition_name)

    def _body(*args):
        operands = list(args)
        if partition_name is not None:
            operands.append(bass2jax.partition_id_tensor())
        outs = bass2jax._bass_exec_p.bind(
            *operands,
            out_avals=tuple(out_avals),
            in_names=tuple(all_in_names),
            out_names=tuple(out_names),
            lowering_input_output_aliases=(),
            sim_require_finite=True,
            sim_require_nnan=True,
            nc=nc,
        )
        return tuple(outs)

    devices = jax.devices()[:n_cores]
    mesh = Mesh(np.asarray(devices), ("core",))
    nspec = n_params + len(out_names)
    fn = jax.jit(
        shard_map(_body, mesh=mesh,
                  in_specs=(PartitionSpec("core"),) * nspec,
                  out_specs=(PartitionSpec("core"),) * len(out_names),
                  check_rep=False),
        keep_unused=True,
    )
    sharding = NamedSharding(mesh, PartitionSpec("core"))
    concat_zero = [
        np.zeros((n_cores * z.shape[0], *z.shape[1:]), z.dtype)
        for z in zero_outs
    ]
    zero_dev = [jax.device_put(a, sharding) for a in concat_zero]
    return dict(fn=fn, in_names=in_names, out_names=out_names,
                out_avals=out_avals, zero_dev=zero_dev, sharding=sharding)


def _run_fast(nc, in_maps, n_cores):
    import jax
    key = id(nc)
    if key not in _RUNNER:
        _RUNNER[key] = _make_runner(nc, n_cores)
    r = _RUNNER[key]
    concat_in = [
        np.concatenate([np.asarray(in_maps[c][name]) for c in range(n_cores)],
                       axis=0)
        for name in r["in_names"]
    ]
    dev_in = [jax.device_put(a, r["sharding"]) for a in concat_in]
    outs = r["fn"](*dev_in, *r["zero_dev"])
    jax.block_until_ready(outs)
    return [
        {name: np.asarray(outs[i]).reshape(n_cores, *r["out_avals"][i].shape)[c]
         for i, name in enumerate(r["out_names"])}
        for c in range(n_cores)
    ]


def kernel(x, batch, W1, b1, W2, b2):
    x = np.asarray(x)
    batch = np.asarray(batch)
    if (x.shape != (262144, D) or batch.shape != (262144,)
            or np.asarray(W1).shape != (D, H)):
        return _numpy_fallback(x, batch, W1, b1, W2, b2)
    if np.any(batch[:-1] > batch[1:]):
        return _numpy_fallback(x, batch, W1, b1, W2, b2)
    b64 = batch.astype(np.int64)
    bounds = np.searchsorted(b64, np.arange(0, N_CORES + 1) * SEGS)
    owned_max = int(np.diff(bounds).max())
    tiles = max(GROUP, -(-owned_max // 128))
    in_maps = make_in_maps(x, batch, W1, b1, W2, b2, tiles=tiles)
    if in_maps is None:
        return _numpy_fallback(x, batch, W1, b1, W2, b2)
    nc = get_nc(tiles)
    try:
        res = _run_fast(nc, in_maps, N_CORES)
        return np.concatenate([res[c]["out"] for c in range(N_CORES)], axis=0)
    except Exception:
        from concourse.bass_utils import run_bass_kernel_spmd
        res = run_bass_kernel_spmd(nc, in_maps, list(range(N_CORES)))
        return np.concatenate(
            [res.results[c]["out"] for c in range(N_CORES)], axis=0)


if __name__ == "__main__":
    pass


# revision 6
# speedup vs baseline: 2.4193x; 1.8882x over previous
"""AttentionPool (segment softmax-pool) Trainium2 kernel, 8 NeuronCores.

Math (reference):
    s = tanh(x @ W1 + b1) @ W2 + b2        # [N,1] scores
    e = exp(s - max(s))                    # global max shift
    out[b] = sum_{i in seg b} e_i x_i / (sum_{i in seg b} e_i + 1e-8)

The global max shift cancels in the ratio (|s| <= ||W2||_1 ~ 9 so exp
never overflows), so e = exp(s) directly.  Batch ids are sorted, so core c
owns segments [128c, 128(c+1)) and processes a fixed window of F rows
starting at the first row of segment 128c.  Rows outside the core's
segments self-mask: their relative id falls outside [0,128) so the
one-hot compare produces zero columns.

Device pipeline (per 16-tile / 4-block group, per core):
  per 512-row block b:
    u   = sum_k W18[:,k].T @ x8[:,k]      # fp8 matmuls, [H, 512] PSUM
    th  = tanh(u + b1)                    # ACT -> bf16
  per group (lagged one group):
    ep4[32b] = w2b.T @ th_b               # 4 col-tiled M=1 matmuls at
                                          # tile_position (0,32b): disjoint
                                          # PE col-groups -> concurrent on HW
    esb = ep4[::32]                       # ONE strided 4-partition DVE copy
    sp  = PE-transpose each [1,128] slice -> [128, gt]  (departition)
    e4  = exp(sp + b2)                    # ACT
    A[t] = (iota == brel[t]) * e4[t]      # DVE one-hot bf16
  per group (lagged two groups), per tile:
    num += A.T @ x_tile                   # fp8 moving, f32 PSUM (260+256)
Final: out = num / (den + 1e-8); host concat across cores.
den rides along as x column 256 == 1.0.

Inputs are host-prepared fp8-e3m4: xb [F, 520] row-major (pass 2) and a
transposed copy xt8 [128, 4, F] (score matmuls).  fp8 error on scores
largely cancels in the softmax ratio; on the pooled sum it stays below
the 2e-2 gate.
"""

import os
import sys

for _p in ("/opt/trn_rl_repo",):
    if os.path.isdir(_p) and _p not in sys.path:
        sys.path.append(_p)

import numpy as np
import ml_dtypes

N_CORES = 8
B = 1024
SEGS = B // N_CORES          # 128 segments owned per core
D = 512
H = 128
F = 33792                    # fixed per-core row window (264 tiles of 128)
TILES = F // 128
GROUP = 16                   # tiles per DMA chunk / departition group
NGROUPS = -(-TILES // GROUP)


PROBE = os.environ.get("KERNEL_PROBE", "")


def build_nc(tiles=TILES, repeats=1, bufs=None, group=GROUP):
    """Build the per-core Bass program. repeats>1 re-emits the whole
    computation for delta-timing."""
    bufs = {**dict(x=3, x8=3, th=8, A=2, esb=2, e4=2, u=2, ep=2, sp=2),
            **(bufs or {})}
    import concourse.bacc as bacc
    import concourse.mybir as mybir
    import concourse.tile as tile

    F32 = mybir.dt.float32
    BF16 = mybir.dt.bfloat16
    FP8 = mybir.dt.float8e3
    AF = mybir.ActivationFunctionType
    OP = mybir.AluOpType

    f_rows = tiles * 128
    ngroups = (tiles + group - 1) // group

    nc = bacc.Bacc(None, target_bir_lowering=False)
    xb_d = nc.dram_tensor("xb", (f_rows, 520), FP8, kind="ExternalInput")
    xt8_d = nc.dram_tensor("xt8", (128, 4, f_rows), FP8,
                           kind="ExternalInput")
    brl_d = nc.dram_tensor("brl", (128, tiles), F32, kind="ExternalInput")
    w18_d = nc.dram_tensor("w18", (128, 4, H), BF16, kind="ExternalInput")
    w2_d = nc.dram_tensor("w2", (H, 1), F32, kind="ExternalInput")
    b1_d = nc.dram_tensor("b1", (H, 1), F32, kind="ExternalInput")
    b2_d = nc.dram_tensor("b2", (128, 1), F32, kind="ExternalInput")
    out_d = nc.dram_tensor("out", (SEGS, D), F32, kind="ExternalOutput")

    import contextlib
    with tile.TileContext(nc) as tc:
        with contextlib.ExitStack() as _stk:
            cpool = _stk.enter_context(tc.tile_pool(name="const", bufs=1))
            xpool = _stk.enter_context(tc.tile_pool(name="xin", bufs=bufs["x"]))
            x8pool = _stk.enter_context(tc.tile_pool(name="x8in", bufs=bufs["x8"]))
            thpool = _stk.enter_context(tc.tile_pool(name="th", bufs=bufs["th"]))
            apool = _stk.enter_context(tc.tile_pool(name="abuild", bufs=bufs["A"]))
            esbpool = _stk.enter_context(tc.tile_pool(name="esb", bufs=bufs["esb"]))
            e4pool = _stk.enter_context(tc.tile_pool(name="e4sb", bufs=bufs["e4"]))
            fpool = _stk.enter_context(tc.tile_pool(name="fin", bufs=1))
            upsum = _stk.enter_context(tc.tile_pool(name="ps_u", bufs=bufs["u"], space="PSUM"))
            eppsum = _stk.enter_context(tc.tile_pool(name="ps_ep", bufs=bufs["ep"], space="PSUM"))
            sppsum = _stk.enter_context(tc.tile_pool(name="ps_sp", bufs=bufs["sp"], space="PSUM"))
            numpsum = _stk.enter_context(tc.tile_pool(name="ps_num", bufs=1, space="PSUM"))
            denpsum = _stk.enter_context(tc.tile_pool(name="ps_den", bufs=1, space="PSUM"))
            # ---- constants ----
            w18 = cpool.tile([128, 4, H], BF16)
            nc.scalar.dma_start(w18[:], w18_d[:])
            w2b = cpool.tile([H, 1], BF16)
            w2f = cpool.tile([H, 1], F32)
            nc.scalar.dma_start(w2f[:], w2_d[:])
            nc.vector.tensor_copy(w2b[:], w2f[:])
            b1s = cpool.tile([H, 1], F32)
            nc.scalar.dma_start(b1s[:], b1_d[:])
            b2s = cpool.tile([128, 1], F32)
            nc.scalar.dma_start(b2s[:], b2_d[:])
            brl = cpool.tile([128, tiles], F32)
            nc.scalar.dma_start(brl[:], brl_d[:])

            ii = cpool.tile([128, 128], mybir.dt.int32)
            nc.gpsimd.iota(ii[:], pattern=[[1, 128]], base=0,
                           channel_multiplier=0)
            iifb = cpool.tile([128, 128], BF16)
            nc.vector.tensor_copy(iifb[:], ii[:])
            i1 = cpool.tile([128, 1], mybir.dt.int32)
            nc.gpsimd.iota(i1[:], pattern=[[0, 1]], base=1,
                           channel_multiplier=0)
            # [128,1] ones in bf16: transpose identity (sliced per block row)
            ones128 = cpool.tile([128, 1], BF16)
            nc.vector.tensor_copy(ones128[:], i1[:])

            numa = numpsum.tile([SEGS, 260], F32)
            numb = denpsum.tile([SEGS, 256], F32)

            for rep in range(repeats):
                # Stage queues (each holds at most one group):
                #   Qep: blocks awaiting the 4 col-tiled ep matmuls + esb copy
                #   Qdp: groups awaiting departition transposes + exp
                #   Qab: groups awaiting A build
                #   Qmm: groups awaiting pass-2 matmuls (depth 2)
                Qep, Qdp, Qab, Qmm = [], [], [], []

                def flush_ep(ent):
                    s, ep4, ths = ent
                    for b, th, nb in ths:
                        nc.tensor.matmul(
                            ep4[32 * b:32 * b + 1, 0:nb],
                            w2b[:], th[:, 0:nb],
                            start=True, stop=True, skip_group_check=True,
                            tile_position=(0, 32 * b),
                        )
                    esb4 = esbpool.tile([128, 512], BF16, tag="esb")
                    full = all(nb == 512 for _, _, nb in ths)
                    if full:
                        # contiguous copy (PSUM forbids partition step>1);
                        # rows between the 32-strided score rows are stale
                        # PSUM and never read downstream
                        nc.vector.tensor_copy(esb4[:, :], ep4[:, :])
                    else:
                        for b, _, nb in ths:
                            nc.vector.tensor_copy(
                                esb4[32 * b:32 * b + 1, 0:nb],
                                ep4[32 * b:32 * b + 1, 0:nb])
                    Qdp.append((s, esb4))

                def flush_dp(ent):
                    s, esb4 = ent
                    gt = min(group, tiles - s * group)
                    sp = sppsum.tile([128, group, 2], BF16, tag="sp")
                    for t in range(gt):
                        j = t // 4
                        c0 = (t % 4) * 128
                        nc.tensor.transpose(
                            sp[:, t, 0:1],
                            esb4[32 * j:32 * j + 1, c0:c0 + 128],
                            ones128[32 * j:32 * j + 1, :],
                            tile_position=(32 * j, 0),
                        )
                    e4sb = e4pool.tile([128, group], F32, tag="e4")
                    nc.scalar.activation(
                        e4sb[:, 0:gt], sp[:, 0:gt, 0],
                        AF.Exp, bias=b2s[:], scale=1.0,
                    )
                    Qab.append((s, e4sb))

                def flush_ab(ent):
                    s, e4sb = ent
                    gt = min(group, tiles - s * group)
                    Ag = apool.tile([128, group, SEGS], BF16, tag="A")
                    for t in range(gt):
                        T = s * group + t
                        nc.vector.tensor_scalar(
                            Ag[:, t, :], iifb[:], brl[:, T:T + 1],
                            e4sb[:, t:t + 1],
                            op0=OP.is_equal, op1=OP.mult,
                        )
                    for i, ent2 in enumerate(Qmm):
                        if ent2[0] == s:
                            Qmm[i] = (s, ent2[1], Ag)

                def flush_mm(ent):
                    s, xs_t, Ag = ent
                    gt = min(group, tiles - s * group)
                    if PROBE == "nonum":
                        return
                    for t in range(gt):
                        T = s * group + t
                        nc.tensor.matmul(
                            numa[:], Ag[:, t, :], xs_t[:, t, 0:260],
                            start=(T == 0), stop=(T == tiles - 1),
                            skip_group_check=True,
                        )
                        nc.tensor.matmul(
                            numb[:], Ag[:, t, :], xs_t[:, t, 260:516],
                            start=(T == 0), stop=(T == tiles - 1),
                            skip_group_check=True,
                        )
                        if PROBE == "2xnum":
                            nc.tensor.matmul(
                                numa[:], Ag[:, t, :], xs_t[:, t, 0:260],
                                start=False, stop=False,
                                skip_group_check=True,
                            )
                            nc.tensor.matmul(
                                numb[:], Ag[:, t, :], xs_t[:, t, 260:516],
                                start=False, stop=False,
                                skip_group_check=True,
                            )

                def load_x8(s, split=1):
                    gt = min(group, tiles - s * group)
                    x8t = x8pool.tile([128, 4, group * 128], FP8, tag="x8")
                    c = gt * 128
                    step = -(-c // split)
                    for lo in range(0, c, step):
                        hi = min(lo + step, c)
                        nc.sync.dma_start(
                            x8t[:, :, lo:hi],
                            xt8_d[:, :, s * group * 128 + lo:
                                  s * group * 128 + hi],
                        )
                    return x8t

                x8next = load_x8(0, split=4)
                for s in range(ngroups):
                    gt = min(group, tiles - s * group)   # tiles this group
                    x8s = x8next
                    if s + 1 < ngroups:
                        x8next = load_x8(s + 1)
                    xs = xpool.tile([128, group, 520], FP8, tag="x")
                    h = (gt + 1) // 2
                    for lo, hi in ((0, h), (h, gt)):
                        if hi > lo:
                            nc.sync.dma_start(
                                xs[:, lo:hi, :],
                                xb_d[(s * group + lo) * 128:
                                     (s * group + hi) * 128, :]
                                .rearrange("(q p) d -> p q d", p=128),
                            )
                    ep4 = eppsum.tile([128, 512], F32, tag="ep")
                    ths = []
                    nblk = (gt * 128 + 511) // 512
                    for b in range(nblk):
                        nb = min(512, gt * 128 - b * 512)
                        u = upsum.tile([H, 512], F32, tag="u")
                        reps_u = 2 if PROBE == "2xu" else 1
                        for k in range(4):
                            for _ru in range(reps_u):
                                nc.tensor.matmul(
                                    u[:, 0:nb],
                                    w18[:, k, :],
                                    x8s[:, k, b * 512:b * 512 + nb],
                                    start=(k == 0 and _ru == 0),
                                    stop=(k == 3 and _ru == reps_u - 1),
                                )
                        th = thpool.tile([H, 512], BF16, tag="th")
                        nc.scalar.activation(
                            th[:, 0:nb], u[:, 0:nb],
                            AF.Tanh, bias=b1s[:], scale=1.0,
                        )
                        ths.append((b, th, nb))
                        if b == 0 and Qep:
                            flush_ep(Qep.pop(0))
                        if b == 1 and Qdp:
                            flush_dp(Qdp.pop(0))
                        if b == 2 and Qab:
                            flush_ab(Qab.pop(0))
                    # partial groups may not hit all flush points
                    if nblk <= 2 and Qab:
                        flush_ab(Qab.pop(0))
                    if nblk <= 1 and Qdp:
                        flush_dp(Qdp.pop(0))
                    Qep.append((s, ep4, ths))
                    Qmm.append((s, xs, None))
                    if len(Qmm) > 2:
                        ent = Qmm.pop(0)
                        assert ent[2] is not None
                        flush_mm(ent)
                # drain
                while Qep:
                    flush_ep(Qep.pop(0))
                while Qdp:
                    flush_dp(Qdp.pop(0))
                while Qab:
                    flush_ab(Qab.pop(0))
                while Qmm:
                    ent = Qmm.pop(0)
                    assert ent[2] is not None
                    flush_mm(ent)

                dsb = fpool.tile([SEGS, 1], F32, tag="dsb")
                nc.vector.tensor_scalar(dsb[:], numa[:, 256:257], 1e-8, None,
                                        op0=OP.add)
                rec = fpool.tile([SEGS, 1], F32, tag="rec")
                nc.vector.reciprocal(rec[:], dsb[:])
                osb = fpool.tile([SEGS, D], F32, tag="osb")
                nc.vector.tensor_scalar(osb[:, 0:256], numa[:, 0:256],
                                        rec[:], None, op0=OP.mult)
                nc.scalar.activation(osb[:, 256:512], numb[:, 0:256],
                                     AF.Copy, scale=rec[:])
                nc.sync.dma_start(out_d[:], osb[:])

    nc.compile()
    return nc


_NC_CACHE = {}


def get_nc(tiles=TILES):
    if tiles not in _NC_CACHE:
        _NC_CACHE[tiles] = build_nc(tiles)
    return _NC_CACHE[tiles]


def make_in_maps(x, batch, W1, b1, W2, b2, tiles=TILES, n_cores=N_CORES):
    """Host-side sharding: segment-aligned fixed windows + relative ids."""
    x = np.ascontiguousarray(np.asarray(x, dtype=np.float32))
    batch = np.asarray(batch).astype(np.int64)
    W1 = np.ascontiguousarray(np.asarray(W1, dtype=np.float32))
    b1 = np.asarray(b1, dtype=np.float32).reshape(H, 1)
    W2 = np.ascontiguousarray(np.asarray(W2, dtype=np.float32).reshape(H, 1))
    b2v = float(np.asarray(b2, dtype=np.float32).reshape(-1)[0])
    b2a = np.full((128, 1), b2v, np.float32)
    fp8 = ml_dtypes.float8_e3m4

    # W18[p, k, h] = W1[k*128 + p, h]
    W18 = np.ascontiguousarray(
        W1.reshape(4, 128, H).transpose(1, 0, 2).astype(ml_dtypes.bfloat16))

    n = x.shape[0]
    f_rows = tiles * 128
    bounds = np.searchsorted(batch, np.arange(0, n_cores + 1) * SEGS)
    owned = np.diff(bounds)
    if owned.max() > f_rows:
        return None  # caller falls back
    pad_to = int(bounds[:-1].max() + f_rows)
    if pad_to > n:
        xp = np.concatenate([x, np.zeros((pad_to - n, D), np.float32)],
                            axis=0)
    else:
        xp = x
    in_maps = []
    for c in range(n_cores):
        o = int(bounds[c])
        xs = xp[o:o + f_rows]
        xa = np.zeros((f_rows, 520), np.float32)
        xa[:, 0:256] = xs[:, 0:256]
        xa[:, 256] = 1.0
        xa[:, 260:516] = xs[:, 256:512]
        xbb = np.ascontiguousarray(xa.astype(fp8))
        # xt8[p, k, c] = x^T[k*128 + p, c]
        xt8 = np.ascontiguousarray(
            xs.T.reshape(4, 128, f_rows).transpose(1, 0, 2).astype(fp8))
        nb = min(f_rows, n - o) if n > o else 0
        br = np.full(f_rows, -1.0, dtype=np.float32)
        br[:nb] = batch[o:o + nb].astype(np.float32) - c * SEGS
        brl2d = np.ascontiguousarray(
            br.reshape(tiles, 128).T)
        in_maps.append({
            "xb": xbb, "xt8": xt8, "brl": brl2d, "w18": W18,
            "w2": W2, "b1": b1, "b2": b2a,
        })
    return in_maps


def _numpy_fallback(x, batch, W1, b1, W2, b2):
    x = np.asarray(x, dtype=np.float32)
    batch = np.asarray(batch).astype(np.int64)
    scores = np.tanh(x @ W1 + b1) @ W2 + b2
    scores = scores - scores.max()
    e = np.exp(scores)
    den = np.zeros((B, 1), np.float32)
    np.add.at(den, batch, e)
    w = e / (den[batch] + 1e-8)
    out = np.zeros((B, D), np.float32)
    np.add.at(out, batch, w * x)
    return out


_RUNNER = {}


def _make_runner(nc, n_cores):
    """Reusable jitted SPMD executable (no donation) so repeated kernel()
    calls skip NEFF/XLA recompilation."""
    import jax
    import concourse.mybir as mybir
    from jax.sharding import Mesh, PartitionSpec, NamedSharding
    from jax.experimental.shard_map import shard_map
    from concourse import bass2jax

    bass2jax.install_neuronx_cc_hook()
    partition_name = (nc.partition_id_tensor.name
                      if nc.partition_id_tensor else None)
    in_names, out_names, out_avals, zero_outs = [], [], [], []
    for alloc in nc.m.functions[0].allocations:
        if not isinstance(alloc, mybir.MemoryLocationSet):
            continue
        name = alloc.memorylocations[0].name
        if alloc.kind == "ExternalInput":
            if name != partition_name:
                in_names.append(name)
        elif alloc.kind == "ExternalOutput":
            shape = tuple(alloc.tensor_shape)
            dtype = mybir.dt.np(alloc.dtype)
            out_names.append(name)
            out_avals.append(jax.core.ShapedArray(shape, dtype))
            zero_outs.append(np.zeros(shape, dtype))
    n_params = len(in_names)
    all_in_names = list(in_names) + list(out_names)
    if partition_name is not None:
        all_in_names.append(partition_name)

    def _body(*args):
        operands = list(args)
        if partition_name is not None:
            operands.append(bass2jax.partition_id_tensor())
        outs = bass2jax._bass_exec_p.bind(
            *operands,
            out_avals=tuple(out_avals),
            in_names=tuple(all_in_names),
            out_names=tuple(out_names),
            lowering_input_output_aliases=(),
            sim_require_finite=True,
            sim_require_nnan=True,
            nc=nc,
        )
        return tuple(outs)

    devices = jax.devices()[:n_cores]
    mesh = Mesh(np.asarray(devices), ("core",))
    nspec = n_params + len(out_names)
    fn = jax.jit(
        shard_map(_body, mesh=mesh,
                  in_specs=(PartitionSpec("core"),) * nspec,
                  out_specs=(PartitionSpec("core"),) * len(out_names),
                  check_rep=False),
        keep_unused=True,
    )
    sharding = NamedSharding(mesh, PartitionSpec("core"))
    concat_zero = [
        np.zeros((n_cores * z.shape[0], *z.shape[1:]), z.dtype)
        for z in zero_outs
    ]
    zero_dev = [jax.device_put(a, sharding) for a in concat_zero]
    return dict(fn=fn, in_names=in_names, out_names=out_names,
                out_avals=out_avals, zero_dev=zero_dev, sharding=sharding)


def _run_fast(nc, in_maps, n_cores):
    import jax
    key = id(nc)
    if key not in _RUNNER:
        _RUNNER[key] = _make_runner(nc, n_cores)
    r = _RUNNER[key]
    concat_in = [
        np.concatenate([np.asarray(in_maps[c][name]) for c in range(n_cores)],
                       axis=0)
        for name in r["in_names"]
    ]
    dev_in = [jax.device_put(a, r["sharding"]) for a in concat_in]
    outs = r["fn"](*dev_in, *r["zero_dev"])
    jax.block_until_ready(outs)
    return [
        {name: np.asarray(outs[i]).reshape(n_cores, *r["out_avals"][i].shape)[c]
         for i, name in enumerate(r["out_names"])}
        for c in range(n_cores)
    ]


def kernel(x, batch, W1, b1, W2, b2):
    x = np.asarray(x)
    batch = np.asarray(batch)
    if (x.shape != (262144, D) or batch.shape != (262144,)
            or np.asarray(W1).shape != (D, H)):
        return _numpy_fallback(x, batch, W1, b1, W2, b2)
    if np.any(batch[:-1] > batch[1:]):
        return _numpy_fallback(x, batch, W1, b1, W2, b2)
    b64 = batch.astype(np.int64)
    bounds = np.searchsorted(b64, np.arange(0, N_CORES + 1) * SEGS)
    owned_max = int(np.diff(bounds).max())
    tiles = max(GROUP, -(-owned_max // 128))
    in_maps = make_in_maps(x, batch, W1, b1, W2, b2, tiles=tiles)
    if in_maps is None:
        return _numpy_fallback(x, batch, W1, b1, W2, b2)
    nc = get_nc(tiles)
    try:
        res = _run_fast(nc, in_maps, N_CORES)
        return np.concatenate([res[c]["out"] for c in range(N_CORES)], axis=0)
    except Exception:
        from concourse.bass_utils import run_bass_kernel_spmd
        res = run_bass_kernel_spmd(nc, in_maps, list(range(N_CORES)))
        return np.concatenate(
            [res.results[c]["out"] for c in range(N_CORES)], axis=0)


if __name__ == "__main__":
    pass


# revision 7
# speedup vs baseline: 38.7050x; 15.9983x over previous
"""AttentionPool (segment softmax-pool) Trainium2 kernel, 8 NeuronCores.

Math (reference):
    s = tanh(x @ W1 + b1) @ W2 + b2        # [N,1] scores
    e = exp(s - max(s))                    # global max shift
    out[b] = sum_{i in seg b} e_i x_i / (sum_{i in seg b} e_i + 1e-8)

The global max shift cancels in the ratio (|s| <= ||W2||_1 ~ 9 so exp
never overflows), so e = exp(s) directly.  Batch ids are sorted, so core c
owns segments [128c, 128(c+1)) and processes a fixed window of F rows
starting at the first row of segment 128c.  Rows outside the core's
segments self-mask: their relative id falls outside [0,128) so the
one-hot compare produces zero columns.

Device pipeline (per 16-tile / 4-block group, per core):
  per 512-row block b:
    u   = sum_k W18[:,k].T @ x8[:,k]      # fp8 matmuls, [H, 512] PSUM
    th  = tanh(u + b1)                    # ACT -> bf16
  per group (lagged one group):
    ep4[32b] = w2b.T @ th_b               # 4 col-tiled M=1 matmuls at
                                          # tile_position (0,32b): disjoint
                                          # PE col-groups -> concurrent on HW
    esb = ep4[::32]                       # ONE strided 4-partition DVE copy
    sp  = PE-transpose each [1,128] slice -> [128, gt]  (departition)
    e4  = exp(sp + b2)                    # ACT
    A[t] = (iota == brel[t]) * e4[t]      # DVE one-hot bf16
  per group (lagged two groups), per tile:
    num += A.T @ x_tile                   # fp8 moving, f32 PSUM (260+256)
Final: out = num / (den + 1e-8); host concat across cores.
den rides along as x column 256 == 1.0.

Inputs are host-prepared fp8-e3m4: xb [F, 520] row-major (pass 2) and a
transposed copy xt8 [128, 4, F] (score matmuls).  fp8 error on scores
largely cancels in the softmax ratio; on the pooled sum it stays below
the 2e-2 gate.
"""

import os
import sys

for _p in ("/opt/trn_rl_repo",):
    if os.path.isdir(_p) and _p not in sys.path:
        sys.path.append(_p)

import numpy as np
import ml_dtypes

N_CORES = 8
B = 1024
SEGS = B // N_CORES          # 128 segments owned per core
D = 512
H = 128
F = 33792                    # fixed per-core row window (264 tiles of 128)
TILES = F // 128
GROUP = 16                   # tiles per DMA chunk / departition group
NGROUPS = -(-TILES // GROUP)


PROBE = os.environ.get("KERNEL_PROBE", "")


def build_nc(tiles=TILES, repeats=1, bufs=None, group=GROUP):
    """Build the per-core Bass program. repeats>1 re-emits the whole
    computation for delta-timing."""
    bufs = {**dict(x=3, x8=3, th=8, A=2, esb=2, e4=2, u=2, ep=2, sp=2),
            **(bufs or {})}
    import concourse.bacc as bacc
    import concourse.mybir as mybir
    import concourse.tile as tile

    F32 = mybir.dt.float32
    BF16 = mybir.dt.bfloat16
    FP8 = mybir.dt.float8e3
    AF = mybir.ActivationFunctionType
    OP = mybir.AluOpType

    f_rows = tiles * 128
    ngroups = (tiles + group - 1) // group

    nc = bacc.Bacc(None, target_bir_lowering=False)
    xb_d = nc.dram_tensor("xb", (f_rows, 520), FP8, kind="ExternalInput")
    xt8_d = nc.dram_tensor("xt8", (128, 4, f_rows), FP8,
                           kind="ExternalInput")
    brl_d = nc.dram_tensor("brl", (128, tiles), F32, kind="ExternalInput")
    w18_d = nc.dram_tensor("w18", (128, 4, H), BF16, kind="ExternalInput")
    w2_d = nc.dram_tensor("w2", (H, 1), F32, kind="ExternalInput")
    b1_d = nc.dram_tensor("b1", (H, 1), F32, kind="ExternalInput")
    b2_d = nc.dram_tensor("b2", (128, 1), F32, kind="ExternalInput")
    out_d = nc.dram_tensor("out", (SEGS, D), F32, kind="ExternalOutput")

    import contextlib
    with tile.TileContext(nc) as tc:
        with contextlib.ExitStack() as _stk:
            cpool = _stk.enter_context(tc.tile_pool(name="const", bufs=1))
            xpool = _stk.enter_context(tc.tile_pool(name="xin", bufs=bufs["x"]))
            x8pool = _stk.enter_context(tc.tile_pool(name="x8in", bufs=bufs["x8"]))
            thpool = _stk.enter_context(tc.tile_pool(name="th", bufs=bufs["th"]))
            apool = _stk.enter_context(tc.tile_pool(name="abuild", bufs=bufs["A"]))
            esbpool = _stk.enter_context(tc.tile_pool(name="esb", bufs=bufs["esb"]))
            e4pool = _stk.enter_context(tc.tile_pool(name="e4sb", bufs=bufs["e4"]))
            fpool = _stk.enter_context(tc.tile_pool(name="fin", bufs=1))
            upsum = _stk.enter_context(tc.tile_pool(name="ps_u", bufs=bufs["u"], space="PSUM"))
            eppsum = _stk.enter_context(tc.tile_pool(name="ps_ep", bufs=bufs["ep"], space="PSUM"))
            sppsum = _stk.enter_context(tc.tile_pool(name="ps_sp", bufs=bufs["sp"], space="PSUM"))
            numpsum = _stk.enter_context(tc.tile_pool(name="ps_num", bufs=1, space="PSUM"))
            denpsum = _stk.enter_context(tc.tile_pool(name="ps_den", bufs=1, space="PSUM"))
            # ---- constants ----
            w18 = cpool.tile([128, 4, H], BF16)
            nc.scalar.dma_start(w18[:], w18_d[:])
            w2b = cpool.tile([H, 1], BF16)
            w2f = cpool.tile([H, 1], F32)
            nc.scalar.dma_start(w2f[:], w2_d[:])
            nc.vector.tensor_copy(w2b[:], w2f[:])
            b1s = cpool.tile([H, 1], F32)
            nc.scalar.dma_start(b1s[:], b1_d[:])
            b2s = cpool.tile([128, 1], F32)
            nc.scalar.dma_start(b2s[:], b2_d[:])
            brl = cpool.tile([128, tiles], F32)
            nc.scalar.dma_start(brl[:], brl_d[:])

            ii = cpool.tile([128, 128], mybir.dt.int32)
            nc.gpsimd.iota(ii[:], pattern=[[1, 128]], base=0,
                           channel_multiplier=0)
            iifb = cpool.tile([128, 128], BF16)
            nc.vector.tensor_copy(iifb[:], ii[:])
            i1 = cpool.tile([128, 1], mybir.dt.int32)
            nc.gpsimd.iota(i1[:], pattern=[[0, 1]], base=1,
                           channel_multiplier=0)
            # [128,1] ones in bf16: transpose identity (sliced per block row)
            ones128 = cpool.tile([128, 1], BF16)
            nc.vector.tensor_copy(ones128[:], i1[:])

            numa = numpsum.tile([SEGS, 260], F32)
            numb = denpsum.tile([SEGS, 256], F32)

            for rep in range(repeats):
                # Stage queues (each holds at most one group):
                #   Qep: blocks awaiting the 4 col-tiled ep matmuls + esb copy
                #   Qdp: groups awaiting departition transposes + exp
                #   Qab: groups awaiting A build
                #   Qmm: groups awaiting pass-2 matmuls (depth 2)
                Qep, Qdp, Qab, Qmm = [], [], [], []

                def flush_ep(ent):
                    s, ep4, ths = ent
                    for b, th, nb in ths:
                        nc.tensor.matmul(
                            ep4[32 * b:32 * b + 1, 0:nb],
                            w2b[:], th[:, 0:nb],
                            start=True, stop=True, skip_group_check=True,
                            tile_position=(0, 32 * b),
                        )
                    esb4 = esbpool.tile([128, 512], BF16, tag="esb")
                    full = all(nb == 512 for _, _, nb in ths)
                    if full:
                        # contiguous copy (PSUM forbids partition step>1);
                        # rows between the 32-strided score rows are stale
                        # PSUM and never read downstream
                        nc.vector.tensor_copy(esb4[:, :], ep4[:, :])
                    else:
                        for b, _, nb in ths:
                            nc.vector.tensor_copy(
                                esb4[32 * b:32 * b + 1, 0:nb],
                                ep4[32 * b:32 * b + 1, 0:nb])
                    Qdp.append((s, esb4))

                def flush_dp(ent):
                    s, esb4 = ent
                    gt = min(group, tiles - s * group)
                    sp = sppsum.tile([128, group, 2], BF16, tag="sp")
                    for t in range(gt):
                        j = t // 4
                        c0 = (t % 4) * 128
                        nc.tensor.transpose(
                            sp[:, t, 0:1],
                            esb4[32 * j:32 * j + 1, c0:c0 + 128],
                            ones128[32 * j:32 * j + 1, :],
                            tile_position=(32 * j, 0),
                        )
                    e4sb = e4pool.tile([128, group], F32, tag="e4")
                    nc.scalar.activation(
                        e4sb[:, 0:gt], sp[:, 0:gt, 0],
                        AF.Exp, bias=b2s[:], scale=1.0,
                    )
                    Qab.append((s, e4sb))

                def flush_ab(ent):
                    s, e4sb = ent
                    gt = min(group, tiles - s * group)
                    Ag = apool.tile([128, group, SEGS], BF16, tag="A")
                    for t in range(gt):
                        T = s * group + t
                        nc.vector.tensor_scalar(
                            Ag[:, t, :], iifb[:], brl[:, T:T + 1],
                            e4sb[:, t:t + 1],
                            op0=OP.is_equal, op1=OP.mult,
                        )
                    for i, ent2 in enumerate(Qmm):
                        if ent2[0] == s:
                            Qmm[i] = (s, ent2[1], Ag)

                def flush_mm(ent):
                    s, xs_t, Ag = ent
                    gt = min(group, tiles - s * group)
                    if PROBE == "nonum":
                        return
                    for t in range(gt):
                        T = s * group + t
                        nc.tensor.matmul(
                            numa[:], Ag[:, t, :], xs_t[:, t, 0:260],
                            start=(T == 0), stop=(T == tiles - 1),
                            skip_group_check=True,
                        )
                        nc.tensor.matmul(
                            numb[:], Ag[:, t, :], xs_t[:, t, 260:516],
                            start=(T == 0), stop=(T == tiles - 1),
                            skip_group_check=True,
                        )
                        if PROBE == "2xnum":
                            nc.tensor.matmul(
                                numa[:], Ag[:, t, :], xs_t[:, t, 0:260],
                                start=False, stop=False,
                                skip_group_check=True,
                            )
                            nc.tensor.matmul(
                                numb[:], Ag[:, t, :], xs_t[:, t, 260:516],
                                start=False, stop=False,
                                skip_group_check=True,
                            )

                def load_x8(s, split=1):
                    gt = min(group, tiles - s * group)
                    x8t = x8pool.tile([128, 4, group * 128], FP8, tag="x8")
                    c = gt * 128
                    step = -(-c // split)
                    for lo in range(0, c, step):
                        hi = min(lo + step, c)
                        nc.sync.dma_start(
                            x8t[:, :, lo:hi],
                            xt8_d[:, :, s * group * 128 + lo:
                                  s * group * 128 + hi],
                        )
                    return x8t

                x8next = load_x8(0, split=4)
                for s in range(ngroups):
                    gt = min(group, tiles - s * group)   # tiles this group
                    x8s = x8next
                    if s + 1 < ngroups:
                        x8next = load_x8(s + 1)
                    xs = xpool.tile([128, group, 520], FP8, tag="x")
                    h = (gt + 1) // 2
                    for lo, hi in ((0, h), (h, gt)):
                        if hi > lo:
                            nc.sync.dma_start(
                                xs[:, lo:hi, :],
                                xb_d[(s * group + lo) * 128:
                                     (s * group + hi) * 128, :]
                                .rearrange("(q p) d -> p q d", p=128),
                            )
                    ep4 = eppsum.tile([128, 512], F32, tag="ep")
                    ths = []
                    nblk = (gt * 128 + 511) // 512
                    for b in range(nblk):
                        nb = min(512, gt * 128 - b * 512)
                        u = upsum.tile([H, 512], F32, tag="u")
                        reps_u = 2 if PROBE == "2xu" else 1
                        for k in range(4):
                            for _ru in range(reps_u):
                                nc.tensor.matmul(
                                    u[:, 0:nb],
                                    w18[:, k, :],
                                    x8s[:, k, b * 512:b * 512 + nb],
                                    start=(k == 0 and _ru == 0),
                                    stop=(k == 3 and _ru == reps_u - 1),
                                )
                        th = thpool.tile([H, 512], BF16, tag="th")
                        nc.scalar.activation(
                            th[:, 0:nb], u[:, 0:nb],
                            AF.Tanh, bias=b1s[:], scale=1.0,
                        )
                        ths.append((b, th, nb))
                        if b == 0 and Qep:
                            flush_ep(Qep.pop(0))
                        if b == 1 and Qdp:
                            flush_dp(Qdp.pop(0))
                        if b == 2 and Qab:
                            flush_ab(Qab.pop(0))
                    # partial groups may not hit all flush points
                    if nblk <= 2 and Qab:
                        flush_ab(Qab.pop(0))
                    if nblk <= 1 and Qdp:
                        flush_dp(Qdp.pop(0))
                    Qep.append((s, ep4, ths))
                    Qmm.append((s, xs, None))
                    # depth 1: group s-1's A tiles were built at block 2 of
                    # this group, so its pass-2 matmuls can flush now —
                    # keeps only one group of matmuls in the serial tail
                    if len(Qmm) > 1 and Qmm[0][2] is not None:
                        flush_mm(Qmm.pop(0))
                # drain
                while Qep:
                    flush_ep(Qep.pop(0))
                while Qdp:
                    flush_dp(Qdp.pop(0))
                while Qab:
                    flush_ab(Qab.pop(0))
                while Qmm:
                    ent = Qmm.pop(0)
                    assert ent[2] is not None
                    flush_mm(ent)

                dsb = fpool.tile([SEGS, 1], F32, tag="dsb")
                nc.vector.tensor_scalar(dsb[:], numa[:, 256:257], 1e-8, None,
                                        op0=OP.add)
                rec = fpool.tile([SEGS, 1], F32, tag="rec")
                nc.vector.reciprocal(rec[:], dsb[:])
                osb = fpool.tile([SEGS, D], F32, tag="osb")
                nc.vector.tensor_scalar(osb[:, 0:256], numa[:, 0:256],
                                        rec[:], None, op0=OP.mult)
                nc.scalar.activation(osb[:, 256:512], numb[:, 0:256],
                                     AF.Copy, scale=rec[:])
                nc.sync.dma_start(out_d[:], osb[:])

    nc.compile()
    return nc


_NC_CACHE = {}


def get_nc(tiles=TILES):
    if tiles not in _NC_CACHE:
        _NC_CACHE[tiles] = build_nc(tiles)
    return _NC_CACHE[tiles]


def make_in_maps(x, batch, W1, b1, W2, b2, tiles=TILES, n_cores=N_CORES):
    """Host-side sharding: segment-aligned fixed windows + relative ids."""
    x = np.ascontiguousarray(np.asarray(x, dtype=np.float32))
    batch = np.asarray(batch).astype(np.int64)
    W1 = np.ascontiguousarray(np.asarray(W1, dtype=np.float32))
    b1 = np.asarray(b1, dtype=np.float32).reshape(H, 1)
    W2 = np.ascontiguousarray(np.asarray(W2, dtype=np.float32).reshape(H, 1))
    b2v = float(np.asarray(b2, dtype=np.float32).reshape(-1)[0])
    b2a = np.full((128, 1), b2v, np.float32)
    fp8 = ml_dtypes.float8_e3m4

    # W18[p, k, h] = W1[k*128 + p, h]
    W18 = np.ascontiguousarray(
        W1.reshape(4, 128, H).transpose(1, 0, 2).astype(ml_dtypes.bfloat16))

    n = x.shape[0]
    f_rows = tiles * 128
    bounds = np.searchsorted(batch, np.arange(0, n_cores + 1) * SEGS)
    owned = np.diff(bounds)
    if owned.max() > f_rows:
        return None  # caller falls back
    pad_to = int(bounds[:-1].max() + f_rows)
    if pad_to > n:
        xp = np.concatenate([x, np.zeros((pad_to - n, D), np.float32)],
                            axis=0)
    else:
        xp = x
    in_maps = []
    for c in range(n_cores):
        o = int(bounds[c])
        xs = xp[o:o + f_rows]
        xa = np.zeros((f_rows, 520), np.float32)
        xa[:, 0:256] = xs[:, 0:256]
        xa[:, 256] = 1.0
        xa[:, 260:516] = xs[:, 256:512]
        xbb = np.ascontiguousarray(xa.astype(fp8))
        # xt8[p, k, c] = x^T[k*128 + p, c]
        xt8 = np.ascontiguousarray(
            xs.T.reshape(4, 128, f_rows).transpose(1, 0, 2).astype(fp8))
        nb = min(f_rows, n - o) if n > o else 0
        br = np.full(f_rows, -1.0, dtype=np.float32)
        br[:nb] = batch[o:o + nb].astype(np.float32) - c * SEGS
        brl2d = np.ascontiguousarray(
            br.reshape(tiles, 128).T)
        in_maps.append({
            "xb": xbb, "xt8": xt8, "brl": brl2d, "w18": W18,
            "w2": W2, "b1": b1, "b2": b2a,
        })
    return in_maps


def _numpy_fallback(x, batch, W1, b1, W2, b2):
    x = np.asarray(x, dtype=np.float32)
    batch = np.asarray(batch).astype(np.int64)
    scores = np.tanh(x @ W1 + b1) @ W2 + b2
    scores = scores - scores.max()
    e = np.exp(scores)
    den = np.zeros((B, 1), np.float32)
    np.add.at(den, batch, e)
    w = e / (den[batch] + 1e-8)
    out = np.zeros((B, D), np.float32)
    np.add.at(out, batch, w * x)
    return out


_RUNNER = {}


def _make_runner(nc, n_cores):
    """Reusable jitted SPMD executable (no donation) so repeated kernel()
    calls skip NEFF/XLA recompilation."""
    import jax
    import concourse.mybir as mybir
    from jax.sharding import Mesh, PartitionSpec, NamedSharding
    from jax.experimental.shard_map import shard_map
    from concourse import bass2jax

    bass2jax.install_neuronx_cc_hook()
    partition_name = (nc.partition_id_tensor.name
                      if nc.partition_id_tensor else None)
    in_names, out_names, out_avals, zero_outs = [], [], [], []
    for alloc in nc.m.functions[0].allocations:
        if not isinstance(alloc, mybir.MemoryLocationSet):
            continue
        name = alloc.memorylocations[0].name
        if alloc.kind == "ExternalInput":
            if name != partition_name:
                in_names.append(name)
        elif alloc.kind == "ExternalOutput":
            shape = tuple(alloc.tensor_shape)
            dtype = mybir.dt.np(alloc.dtype)
            out_names.append(name)
            out_avals.append(jax.core.ShapedArray(shape, dtype))
            zero_outs.append(np.zeros(shape, dtype))
    n_params = len(in_names)
    all_in_names = list(in_names) + list(out_names)
    if partition_name is not None:
        all_in_names.append(partition_name)

    def _body(*args):
        operands = list(args)
        if partition_name is not None:
            operands.append(bass2jax.partition_id_tensor())
        outs = bass2jax._bass_exec_p.bind(
            *operands,
            out_avals=tuple(out_avals),
            in_names=tuple(all_in_names),
            out_names=tuple(out_names),
            lowering_input_output_aliases=(),
            sim_require_finite=True,
            sim_require_nnan=True,
            nc=nc,
        )
        return tuple(outs)

    devices = jax.devices()[:n_cores]
    mesh = Mesh(np.asarray(devices), ("core",))
    nspec = n_params + len(out_names)
    fn = jax.jit(
        shard_map(_body, mesh=mesh,
                  in_specs=(PartitionSpec("core"),) * nspec,
                  out_specs=(PartitionSpec("core"),) * len(out_names),
                  check_rep=False),
        keep_unused=True,
    )
    sharding = NamedSharding(mesh, PartitionSpec("core"))
    concat_zero = [
        np.zeros((n_cores * z.shape[0], *z.shape[1:]), z.dtype)
        for z in zero_outs
    ]
    zero_dev = [jax.device_put(a, sharding) for a in concat_zero]
    return dict(fn=fn, in_names=in_names, out_names=out_names,
                out_avals=out_avals, zero_dev=zero_dev, sharding=sharding)


def _run_fast(nc, in_maps, n_cores):
    import jax
    key = id(nc)
    if key not in _RUNNER:
        _RUNNER[key] = _make_runner(nc, n_cores)
    r = _RUNNER[key]
    concat_in = [
        np.concatenate([np.asarray(in_maps[c][name]) for c in range(n_cores)],
                       axis=0)
        for name in r["in_names"]
    ]
    dev_in = [jax.device_put(a, r["sharding"]) for a in concat_in]
    outs = r["fn"](*dev_in, *r["zero_dev"])
    jax.block_until_ready(outs)
    return [
        {name: np.asarray(outs[i]).reshape(n_cores, *r["out_avals"][i].shape)[c]
         for i, name in enumerate(r["out_names"])}
        for c in range(n_cores)
    ]


def kernel(x, batch, W1, b1, W2, b2):
    x = np.asarray(x)
    batch = np.asarray(batch)
    if (x.shape != (262144, D) or batch.shape != (262144,)
            or np.asarray(W1).shape != (D, H)):
        return _numpy_fallback(x, batch, W1, b1, W2, b2)
    if np.any(batch[:-1] > batch[1:]):
        return _numpy_fallback(x, batch, W1, b1, W2, b2)
    b64 = batch.astype(np.int64)
    bounds = np.searchsorted(b64, np.arange(0, N_CORES + 1) * SEGS)
    owned_max = int(np.diff(bounds).max())
    tiles = max(GROUP, -(-owned_max // 128))
    in_maps = make_in_maps(x, batch, W1, b1, W2, b2, tiles=tiles)
    if in_maps is None:
        return _numpy_fallback(x, batch, W1, b1, W2, b2)
    nc = get_nc(tiles)
    try:
        res = _run_fast(nc, in_maps, N_CORES)
        return np.concatenate([res[c]["out"] for c in range(N_CORES)], axis=0)
    except Exception:
        from concourse.bass_utils import run_bass_kernel_spmd
        res = run_bass_kernel_spmd(nc, in_maps, list(range(N_CORES)))
        return np.concatenate(
            [res.results[c]["out"] for c in range(N_CORES)], axis=0)


if __name__ == "__main__":
    pass
